# revision 1
# baseline (speedup 1.0000x reference)
"""CustomPoseLoss Trainium2 kernel.

loss = mean((pred-target)^2) + 0.5 * mean((R(pred)-R(target))^2)
where R(M) = sign(det M) * polar(M) for each 3x3 matrix (row of 9).

Implementation: closed-form polar decomposition per row, fully vectorized as
channel-plane arithmetic on the Vector/Scalar engines:
  S = M^T M, normalized by tr(S)/3; eigenvalues of S via Cardano
  (acos/cos evaluated as polynomials so only the sqrt LUT set is needed);
  W^-1 = (S + s2 I) adj(N) / det(N) with N = ssig*S + Pg*I  (Cayley-Hamilton
  inverse-sqrt);  R = sign(det) * M W^-1 / sqrt(m).
det(N) is formed from the eigenvalue product (positive, cancellation-free)
and clamped, so near-singular rows stay bounded.

Sharding: pure data parallel over 8 cores; each core reduces its shard to
[128, 2, NCHUNK] partial sums (mse, rot), host combines in float64.
"""

import numpy as np

B = 1048576
N_CORES = 8
ROWS_PER_CORE = B // N_CORES          # 131072
P = 128
ROWS_PER_PART = ROWS_PER_CORE // P    # 1024
T = 256                               # rows per partition per chunk
NCHUNK = ROWS_PER_PART // T           # 4
EPS_D = 1e-5

ACOS_A = (1.5707288, -0.2121144, 0.0742610, -0.0187293)   # A&S 4.4.45
HALF_SQRT3 = 0.8660254037844386


class Regs:
    """[128, 2, T] fp32 plane slots with explicit reuse (SBUF is capped)."""

    def __init__(self, pool, dtype, prefix="v", shape=None):
        self.pool = pool
        self.dtype = dtype
        self.prefix = prefix
        self.shape = shape or [P, 2 * T]
        self.free_tags = []
        self.n = 0
        self.tag_of = {}

    def alloc(self):
        if self.free_tags:
            tag = self.free_tags.pop()
        else:
            self.n += 1
            tag = f"{self.prefix}{self.n}"
        tl = self.pool.tile(self.shape, self.dtype, tag=tag)
        self.tag_of[id(tl)] = tag
        return tl

    def free(self, *tiles):
        for tl in tiles:
            self.free_tags.append(self.tag_of.pop(id(tl)))


LN3 = float(np.log(3.0))
LN6 = float(np.log(6.0))
LN2 = float(np.log(2.0))
EPS_W = 6e-3


def _build_chunk(nc, regs, regs16, praw, traw, acc_mse_col, acc_rot_col,
                 X, D, Sm, QS, Nm, Am, T1m, RT, dR, bias0, mybir):
    Alu = mybir.AluOpType
    Act = mybir.ActivationFunctionType
    L = 2 * T

    def mul(o, a, b):
        nc.vector.tensor_tensor(out=o, in0=a, in1=b, op=Alu.mult)

    def add(o, a, b):
        nc.vector.tensor_tensor(out=o, in0=a, in1=b, op=Alu.add)

    def sub(o, a, b):
        nc.vector.tensor_tensor(out=o, in0=a, in1=b, op=Alu.subtract)

    def vs(o, a, s1, op0, s2=None, op1=None):
        if s2 is None:
            nc.vector.tensor_scalar(out=o, in0=a, scalar1=float(s1),
                                    scalar2=None, op0=getattr(Alu, op0))
        else:
            nc.vector.tensor_scalar(out=o, in0=a, scalar1=float(s1),
                                    scalar2=float(s2), op0=getattr(Alu, op0),
                                    op1=getattr(Alu, op1))

    def stt(o, a, s, b, op0, op1):
        nc.vector.scalar_tensor_tensor(out=o, in0=a, scalar=float(s), in1=b,
                                       op0=getattr(Alu, op0),
                                       op1=getattr(Alu, op1))

    def act(o, a, func, scale=1.0, bias=None, accum_out=None):
        if func == "Copy":
            nc.scalar.activation(out=o, in_=a, func=Act.Copy, bias=0.0,
                                 scale=float(scale), accum_out=accum_out)
        else:
            nc.scalar.activation(out=o, in_=a, func=getattr(Act, func),
                                 bias=bias0[:, 0:1] if bias is None else bias,
                                 scale=float(scale), accum_out=accum_out)

    def bc(plane, k):
        # broadcast [P, L] plane across k sub-planes -> [P, k, L]
        return bass_mod.AP(tensor=plane.tensor, offset=plane.offset,
                           ap=[plane.ap[0], [0, k], plane.ap[1]])

    A = regs.alloc          # fp32 [P, L] planes
    H = regs16.alloc        # fp16 [P, L] planes

    # ---- cast+deinterleave both inputs into X[P, 9, 2T] (f16) ----
    rvp = praw.rearrange("p (n c) -> p n c", c=9)
    rvt = traw.rearrange("p (n c) -> p n c", c=9)
    xin_p = bass_mod.AP(tensor=rvp.tensor, offset=rvp.offset,
                        ap=[rvp.ap[0], rvp.ap[2], rvp.ap[1]])
    xin_t = bass_mod.AP(tensor=rvt.tensor, offset=rvt.offset,
                        ap=[rvt.ap[0], rvt.ap[2], rvt.ap[1]])
    act(X[:, :, 0:T], xin_p, "Copy")
    act(X[:, :, T:L], xin_t, "Copy")
    x = [X[:, c, :] for c in range(9)]          # [P, L] f16 unit-stride

    # ---- mse: D = pred - target (f16), accum sum(D^2) on ACT ----
    sub(D, X[:, :, 0:T], X[:, :, T:L])
    Df = D.rearrange("p c n -> p (c n)")
    act(Df, Df, "Square", accum_out=acc_mse_col)

    # ---- S = M^T M (f16): order [s00,s11,s22,s01,s02,s12] ----
    tmp16 = H()
    for i in range(3):
        sd = Sm[:, i, :]
        mul(sd, x[i], x[i])
        mul(tmp16, x[i+3], x[i+3]); add(sd, sd, tmp16)
        mul(tmp16, x[i+6], x[i+6]); add(sd, sd, tmp16)
    for oi, (ia, ib) in enumerate((((0,3,6),(1,4,7)), ((0,3,6),(2,5,8)),
                                   ((1,4,7),(2,5,8)))):
        so = Sm[:, 3+oi, :]
        mul(so, x[ia[0]], x[ib[0]])
        mul(tmp16, x[ia[1]], x[ib[1]]); add(so, so, tmp16)
        mul(tmp16, x[ia[2]], x[ib[2]]); add(so, so, tmp16)
    # tr and normalization scale q = 3/tr (ln domain)
    tr16 = H()
    add(tr16, Sm[:, 0, :], Sm[:, 1, :]); add(tr16, tr16, Sm[:, 2, :])
    vs(tr16, tr16, 6e-5, "max")
    lnt = A(); act(lnt, tr16, "Ln")
    q16 = H(); act(q16, lnt, "Exp", scale=-1.0, bias=_c(nc, LN3))
    regs16.free(tr16)
    nc.vector.tensor_tensor(out=Sm[:, :, :], in0=Sm[:, :, :], in1=bc(q16, 6),
                            op=Alu.mult)
    regs16.free(q16)

    # ---- det(M) fp32 from raw (strided channel views) ----
    xr = [None] * 9
    for c in range(9):
        ap_p = rvp[:, :, c]
        ap_t = rvt[:, :, c]
        xr[c] = (ap_p, ap_t)
    cA, cB, det, tmpd = A(), A(), A(), A()
    def rmul(o, i, j):
        # o[:, :T] = pred_ch_i*pred_ch_j ; o[:, T:] = target halves
        mul(o[:, 0:T], xr[i][0], xr[j][0])
        mul(o[:, T:L], xr[i][1], xr[j][1])
    def rmul2(o, i, co):
        mul(o[:, 0:T], xr[i][0], co[:, 0:T])
        mul(o[:, T:L], xr[i][1], co[:, T:L])
    rmul(cA, 4, 8); rmul(cB, 5, 7); sub(cA, cA, cB)
    rmul2(det, 0, cA)
    rmul(cA, 5, 6); rmul(cB, 3, 8); sub(cA, cA, cB)
    rmul2(tmpd, 1, cA); add(det, det, tmpd)
    rmul(cA, 3, 7); rmul(cB, 4, 6); sub(cA, cA, cB)
    rmul2(tmpd, 2, cA); add(det, det, tmpd)
    regs.free(cA)
    sgd = A(); act(sgd, det, "Sign")
    lnad = A(); act(cB, det, "Abs")
    act(lnad, cB, "Ln")
    regs.free(cB, det, tmpd)
    # Pg = exp(lnad + 1.5*(ln3 - lnt));  dets = Pg^2
    lnPg = A()
    stt(lnPg, lnt, -1.5, lnad, "mult", "add")
    regs.free(lnad)
    Pg32 = A(); act(Pg32, lnPg, "Exp", scale=1.0, bias=_c(nc, 1.5 * LN3))
    Pg16 = H(); act(Pg16, Pg32, "Copy")
    dets = A(); act(dets, Pg32, "Square")
    regs.free(lnPg)

    # ---- tr(S^2) fp32 from normalized f16 S ----
    act(QS, Sm, "Square")
    u1, u2 = A(), A()
    add(u1, QS[:, 0, :], QS[:, 1, :]); add(u1, u1, QS[:, 2, :])
    add(u2, QS[:, 3, :], QS[:, 4, :]); add(u2, u2, QS[:, 5, :])
    trS2 = A()
    stt(trS2, u2, 2.0, u1, "mult", "add")
    # p and 1/(2 p^3) via ln/exp
    trK2, p, ip3h = u1, A(), u2            # reuse u1/u2 slots
    vs(trK2, trS2, -3.0, "add", 1e-30, "max")
    lnk = A(); act(lnk, trK2, "Ln")
    act(p, lnk, "Exp", scale=0.5, bias=_c(nc, -0.5 * LN6))
    act(ip3h, lnk, "Exp", scale=-1.5, bias=_c(nc, 1.5 * LN6 - LN2))
    vs(ip3h, ip3h, 1e30, "min")
    regs.free(lnk)
    # arg
    detK, arg = A(), A()
    stt(detK, trS2, 0.5, dets, "mult", "add")
    vs(detK, detK, -2.5, "add")
    mul(arg, detK, ip3h)
    vs(arg, arg, 1.0, "min", -1.0, "max")
    regs.free(detK, trS2, dets, u2)   # u2 == ip3h
    # ---- th3 = acos(arg) ----
    y, om, h = A(), A(), A()
    act(y, arg, "Abs")
    vs(om, y, -1.0, "mult", 1.0, "add")
    lnom = A(); act(lnom, om, "Ln")
    act(om, lnom, "Exp", scale=0.5)              # sqrt(1-y)
    regs.free(lnom)
    vs(h, y, ACOS_A[3], "mult", ACOS_A[2], "add")
    mul(h, h, y); vs(h, h, ACOS_A[1], "add")
    mul(h, h, y); vs(h, h, ACOS_A[0], "add")
    mul(h, h, om)
    sg_a, th3 = y, om                            # reuse slots
    act(sg_a, arg, "Sign")
    vs(arg, sg_a, -np.pi/2, "mult", np.pi/2, "add")
    mul(th3, sg_a, h); add(th3, th3, arg)
    regs.free(h, arg, y)    # th3 == om stays
    # ---- cos((th3+2pik)/3) ----
    z, c0p = A(), A()
    act(z, th3, "Square", scale=1.0/3.0)
    vs(c0p, z, 1.0/40320.0, "mult", -1.0/720.0, "add")
    mul(c0p, c0p, z); vs(c0p, c0p, 1.0/24.0, "add")
    mul(c0p, c0p, z); vs(c0p, c0p, -0.5, "add")
    mul(c0p, c0p, z); vs(c0p, c0p, 1.0, "add")
    regs.free(z, om)   # om == th3
    s0, uc1, c1p, c2p = A(), A(), A(), A()
    act(s0, c0p, "Square")
    vs(s0, s0, -1.0, "mult", 1.0, "add")
    vs(s0, s0, 0.0, "max")
    lns = A(); act(lns, s0, "Ln")
    act(s0, lns, "Exp", scale=0.5)
    regs.free(lns)
    vs(uc1, c0p, -0.5, "mult")
    vs(s0, s0, HALF_SQRT3, "mult")
    sub(c1p, uc1, s0)
    add(c2p, uc1, s0)
    regs.free(s0, uc1)
    # ---- lambda_k, g_k = sqrt(lambda_k) ----
    tp = A()
    vs(tp, p, 2.0, "mult")
    regs.free(p)
    lam, g = [], []
    for ck in (c0p, c1p, c2p):
        lk, gk, lnl = A(), A(), A()
        mul(lk, tp, ck)
        vs(lk, lk, 1.0, "add", 1e-35, "max")
        act(lnl, lk, "Ln")
        act(gk, lnl, "Exp", scale=0.5)
        regs.free(lnl)
        lam.append(lk); g.append(gk)
    regs.free(tp, c0p, c1p, c2p)
    g01, ssig, s2i, tmp2 = A(), A(), A(), A()
    add(g01, g[0], g[1])
    add(ssig, g01, g[2])
    mul(s2i, g[0], g[1]); mul(tmp2, g[2], g01); add(s2i, s2i, tmp2)
    regs.free(g01, tmp2, *g)
    # ---- w = sign/(sqrt(m)*detN) via ln domain, clamped ----
    t_, nuk, lnn = A(), A(), A()
    mul(nuk, ssig, lam[0]); add(nuk, nuk, Pg32)
    act(t_, nuk, "Ln")
    mul(nuk, ssig, lam[1]); add(nuk, nuk, Pg32)
    act(lnn, nuk, "Ln"); add(t_, t_, lnn)
    mul(nuk, ssig, lam[2]); add(nuk, nuk, Pg32)
    act(lnn, nuk, "Ln"); add(t_, t_, lnn)
    stt(t_, lnt, 0.5, t_, "mult", "add")
    vs(t_, t_, float(np.log(EPS_W) + 0.5*LN3), "max")
    wmag = lnn                                  # reuse
    act(wmag, t_, "Exp", scale=-1.0, bias=_c(nc, 0.5 * LN3))
    w32 = A()
    mul(w32, wmag, sgd)
    regs.free(t_, nuk, lnn, sgd, lnt, *lam)
    ssig16, s2i16 = H(), H()
    act(ssig16, ssig, "Copy")
    act(s2i16, s2i, "Copy")
    regs.free(ssig, s2i, Pg32)

    # ---- N = ssig*S + Pg*I (f16, batched) ----
    nc.vector.tensor_tensor(out=Nm[:, :, :], in0=Sm[:, :, :],
                            in1=bc(ssig16, 6), op=Alu.mult)
    nc.vector.tensor_tensor(out=Nm[:, 0:3, :], in0=Nm[:, 0:3, :],
                            in1=bc(Pg16, 3), op=Alu.add)
    regs16.free(Pg16, ssig16)
    # A' diagonal (Am = S_diag + s2)
    nc.vector.tensor_tensor(out=Am[:, :, :], in0=Sm[:, 0:3, :],
                            in1=bc(s2i16, 3), op=Alu.add)
    regs16.free(s2i16)
    # ---- adj(N) (f16) -> stored into QS? no: reuse Nm? need both. use T1m? no.
    n00, n11, n22 = (Nm[:, i, :] for i in range(3))
    n01, n02, n12 = (Nm[:, i, :] for i in range(3, 6))
    aj = [H() for _ in range(6)]
    a00, a01, a02, a11, a12, a22 = aj
    def cof(o, a, b, c, dd):
        mul(o, a, b); mul(tmp16, c, dd); sub(o, o, tmp16)
    cof(a00, n11, n22, n12, n12)
    cof(a01, n02, n12, n01, n22)
    cof(a02, n01, n12, n02, n11)
    cof(a11, n00, n22, n02, n02)
    cof(a12, n01, n02, n00, n12)
    cof(a22, n00, n11, n01, n01)
    # ---- T1 = A' adjN (f16); rows of A': (b0,s01,s02),(s01,b1,s12),(s02,s12,b2)
    b0, b1, b2 = (Am[:, i, :] for i in range(3))
    s01p, s02p, s12p = Sm[:, 3, :], Sm[:, 4, :], Sm[:, 5, :]
    def mm3(o, r0, r1, r2, k0, k1, k2):
        mul(o, r0, k0)
        mul(tmp16, r1, k1); add(o, o, tmp16)
        mul(tmp16, r2, k2); add(o, o, tmp16)
    mm3(T1m[:, 0, :], b0, s01p, s02p, a00, a01, a02)
    mm3(T1m[:, 1, :], b0, s01p, s02p, a01, a11, a12)
    mm3(T1m[:, 2, :], b0, s01p, s02p, a02, a12, a22)
    mm3(T1m[:, 3, :], s01p, b1, s12p, a01, a11, a12)
    mm3(T1m[:, 4, :], s01p, b1, s12p, a02, a12, a22)
    mm3(T1m[:, 5, :], s02p, s12p, b2, a02, a12, a22)
    regs16.free(*aj)
    # ---- T2 = w*T1 in fp32 (QS tile is dead; reuse it) ----
    T2f = QS
    nc.vector.tensor_tensor(out=T2f[:, :, :], in0=T1m[:, :, :],
                            in1=bc(w32, 6), op=Alu.mult)
    regs.free(w32)
    t00, t01, t02 = T2f[:, 0, :], T2f[:, 1, :], T2f[:, 2, :]
    t11, t12, t22 = T2f[:, 3, :], T2f[:, 4, :], T2f[:, 5, :]
    T2 = [[t00, t01, t02], [t01, t11, t12], [t02, t12, t22]]
    # ---- R = M*T2 (fp32 out), clamp junk rows, dR, accumulate ----
    tmpr = regs.alloc()
    for i in range(3):
        for j in range(3):
            o = RT[:, 3*i+j, :]
            mul(o, x[3*i], T2[0][j])
            mul(tmpr, x[3*i+1], T2[1][j]); add(o, o, tmpr)
            mul(tmpr, x[3*i+2], T2[2][j]); add(o, o, tmpr)
    regs.free(tmpr)
    RTf = RT.rearrange("p c n -> p (c n)")
    nc.vector.tensor_scalar(out=RTf, in0=RTf, scalar1=8.0, scalar2=-8.0,
                            op0=Alu.min, op1=Alu.max)
    sub(dR, RT[:, :, 0:T], RT[:, :, T:L])
    dRf = dR.rearrange("p c n -> p (c n)")
    act(dRf, dRf, "Square", accum_out=acc_rot_col)
    regs16.free(tmp16)



_CONST_STATE = {}
bass_mod = None


def _c(nc, v):
    """[P,1] fp32 constant AP, DVE-memset once (keeps ACT single-wait)."""
    key = float(np.float32(v))
    consts = _CONST_STATE.setdefault(id(nc), {})
    if key not in consts:
        pool = _CONST_STATE[(id(nc), "pool")]
        from concourse import mybir
        t = pool.tile([P, 1], mybir.dt.float32, tag=f"c{len(consts)}")
        nc.vector.memset(t, key)
        consts[key] = t
    return consts[key][:, 0:1]


def _build_nc():
    global bass_mod
    import concourse.bass as bass
    import concourse.tile as tile
    from concourse import mybir
    bass_mod = bass

    f32 = mybir.dt.float32
    f16 = mybir.dt.float16
    nc = bass.Bass()
    pred = nc.dram_tensor("pred", [ROWS_PER_CORE, 9], f32, kind="ExternalInput")
    targ = nc.dram_tensor("target", [ROWS_PER_CORE, 9], f32, kind="ExternalInput")
    out = nc.dram_tensor("partials", [P, 2 * NCHUNK], f32, kind="ExternalOutput")

    predv = pred.rearrange("(p n) c -> p n c", p=P)    # [128, 1024, 9]
    targv = targ.rearrange("(p n) c -> p n c", p=P)

    with tile.TileContext(nc) as tc:
        with (
            tc.tile_pool(name="raw", bufs=1) as rawp,
            tc.tile_pool(name="pl", bufs=1) as pl,
            tc.tile_pool(name="acc", bufs=1) as accp,
        ):
            acc = accp.tile([P, 2 * NCHUNK], f32, tag="acc")
            bias0 = accp.tile([P, 1], f32, tag="bias0")
            nc.vector.memset(bias0, 0.0)
            _CONST_STATE[(id(nc), "pool")] = accp
            regs = Regs(pl, f32, prefix="v", shape=[P, 2 * T])
            regs16 = Regs(pl, f16, prefix="h", shape=[P, 2 * T])
            L = 2 * T
            praw_all = rawp.tile([P, ROWS_PER_PART * 9], f32, tag="praw")
            traw_all = rawp.tile([P, ROWS_PER_PART * 9], f32, tag="traw")
            # two-piece load: chunk-0 compute overlaps the bulk transfer
            nc.sync.dma_start(out=praw_all[:, 0:T*9], in_=predv[:, 0:T, :])
            nc.sync.dma_start(out=traw_all[:, 0:T*9], in_=targv[:, 0:T, :])
            nc.sync.dma_start(out=praw_all[:, T*9:], in_=predv[:, T:, :])
            nc.sync.dma_start(out=traw_all[:, T*9:], in_=targv[:, T:, :])
            for k in range(NCHUNK):
                praw = praw_all[:, k*T*9:(k+1)*T*9]
                traw = traw_all[:, k*T*9:(k+1)*T*9]
                X = pl.tile([P, 9, L], f16, tag=f"X{k%2}")
                D = pl.tile([P, 9, T], f16, tag="D")
                Sm = pl.tile([P, 6, L], f16, tag=f"Sm{k%2}")
                QS = pl.tile([P, 6, L], f32, tag="QS")
                Nm = pl.tile([P, 6, L], f16, tag="Nm")
                Am = pl.tile([P, 3, L], f16, tag="Am")
                T1m = pl.tile([P, 6, L], f16, tag="T1m")
                RT = pl.tile([P, 9, L], f32, tag="RT")
                dR = pl.tile([P, 9, T], f32, tag="dRt")
                _build_chunk(nc, regs, regs16, praw, traw,
                             acc[:, k:k+1], acc[:, NCHUNK+k:NCHUNK+k+1],
                             X, D, Sm, QS, Nm, Am, T1m, RT, dR, bias0, mybir)
            nc.sync.dma_start(out=out[:, :], in_=acc)
    return nc


def _elide_implied_waits(nc):
    """Drop semaphore waits already implied by program order or transitively
    by earlier waits (vector-clock propagation).  Tile's per-instruction wait
    emission is not transitively minimal, and walrus can encode only one sync
    wait on Activation/DMA instructions (and ~4 on control instructions), so
    the redundant waits both break codegen and waste sequencer time.

    Model: each semaphore s carries a snapshot VC at every increment value;
    an engine's observed VC advances via its own instruction stream and via
    the snapshots of the waits it executes.  A wait (s >= v) is dropped iff
    the engine's observed VC already dominates it.  Unknown update modes
    disable elision for that semaphore (conservative).
    """
    join = lambda a, b: {k: max(a.get(k, 0), b.get(k, 0)) for k in set(a) | set(b)}
    sem_val = {}        # sem name -> current value
    sem_snap = {}       # sem name -> list of (value, VC) snapshots
    eng_vc = {}         # engine name -> observed VC
    unsafe = set()      # sems with non-increment updates
    n_drop = 0
    for f in nc.m.functions:
        for bb in f.blocks:
            for ins in bb.instructions:
                eng = str(ins.engine)
                vc = dict(eng_vc.get(eng, {}))
                si = ins.sync_info
                waits = list(si.on_wait) if si is not None and si.on_wait else []
                kept = []
                for w in waits:
                    s, v = w.ant_name, w.wait_value
                    if w.wait_mode != "sem-ge-imm" or s in unsafe:
                        kept.append(w)
                        continue
                    if vc.get(s, 0) >= v:
                        n_drop += 1
                        continue
                    if sem_val.get(s, 0) < v:
                        # increment not yet seen in emission order; keep and
                        # learn nothing (conservative)
                        kept.append(w)
                        continue
                    kept.append(w)
                    snap = {}
                    for sv, svc in sem_snap.get(s, ()):
                        if sv <= v:
                            snap = svc
                        else:
                            break
                    vc = join(vc, snap)
                    vc[s] = max(vc.get(s, 0), v)
                if si is not None and len(kept) != len(waits):
                    si.on_wait = kept
                # apply this instruction's increments
                ups = si.on_update if si is not None and si.on_update else []
                for u in ups:
                    s = u.ant_name
                    if u.update_mode not in ("sem-inc", "sem-add-imm"):
                        unsafe.add(s)
                        continue
                    nv = sem_val.get(s, 0) + (u.update_value or 1)
                    sem_val[s] = nv
                    lst = sem_snap.setdefault(s, [])
                    prev = lst[-1][1] if lst else {}
                    lst.append((nv, join(prev, vc)))
                    # Engine-sem increments fire when the instruction
                    # completes, and the engine is sequential, so later
                    # instructions on this engine observe them.  DMA-queue
                    # increments fire asynchronously at transfer completion:
                    # the issuing engine must NOT absorb those.
                    if "DMA" not in s:
                        vc[s] = max(vc.get(s, 0), nv)
                eng_vc[eng] = vc
    return n_drop


_NC_CACHE = None


def kernel(pred: np.ndarray, target: np.ndarray) -> np.ndarray:
    global _NC_CACHE
    from concourse.bass_utils import run_bass_kernel_spmd

    pred = np.ascontiguousarray(np.asarray(pred, dtype=np.float32))
    target = np.ascontiguousarray(np.asarray(target, dtype=np.float32))
    assert pred.shape == (B, 9) and target.shape == (B, 9)

    if _NC_CACHE is None:
        _NC_CACHE = _build_nc()
        _elide_implied_waits(_NC_CACHE)
    nc = _NC_CACHE

    ps = pred.reshape(N_CORES, ROWS_PER_CORE, 9)
    ts = target.reshape(N_CORES, ROWS_PER_CORE, 9)
    in_maps = [{"pred": ps[i], "target": ts[i]} for i in range(N_CORES)]
    res = run_bass_kernel_spmd(nc, in_maps, core_ids=list(range(N_CORES)))
    globals()["_LAST_RESULT"] = res

    mse_sum = 0.0
    rot_sum = 0.0
    for r in res.results:
        part = np.asarray(r["partials"], dtype=np.float64)
        mse_sum += part[:, :NCHUNK].sum()
        rot_sum += part[:, NCHUNK:].sum()
    n = float(B * 9)
    return np.asarray(np.float32(mse_sum / n + 0.5 * (rot_sum / n)))



# revision 8
# speedup vs baseline: 1.8315x; 1.8315x over previous
"""CustomPoseLoss Trainium2 kernel.

loss = mean((pred-target)^2) + 0.5 * mean((R(pred)-R(target))^2)
where R(M) = sign(det M) * polar(M) for each 3x3 matrix (row of 9).

Implementation: det-scaled Newton iteration for the polar factor, with the
sign fix folded into the first iteration's scaling (R = polar(sign(det M)*M),
and the signed cube root sign(d)*|d|^{-1/3} handles it for free):

  Z_0 = M
  Z_{k+1} = a_k * Z_k + b_k * cof(Z_k),   a = d^{-1/3}, b = d^{-2/3}
  (d = det Z_k; the 0.5 Newton averaging is deferred into the scaling and
   applied via a ln(0.5) bias on the final iteration's exponentials)

All plane arithmetic is f16 unit-stride so DVE tensor_tensor runs in 2x mode;
det is accumulated in fp32 (avoids f16 inf -> NaN); iterates are clamped to
+-180 before each cofactor pass so every f16 product stays below 65504.
The transcendental chain (Square/Ln/Exp/Sign) runs on the Scalar engine in
parallel with the Vector engine's cofactor work of the other chunk
(two chunks are software-pipelined for exactly this overlap).

Sharding: pure data parallel over 8 cores; each core reduces its shard to
[128, 2*NCHUNK] partial sums (mse, rot), host combines in float64.
"""

import numpy as np

B = 1048576
N_CORES = 8
ROWS_PER_CORE = B // N_CORES          # 131072
P = 128
ROWS_PER_PART = ROWS_PER_CORE // P    # 1024
T = 512                               # rows per partition per chunk (per tensor)
NCHUNK = ROWS_PER_PART // T           # 2
L = 2 * T                             # plane width: [pred rows | target rows]
K_ITERS = 3
CLAMP_IT = 180.0
EPS_D = 1e-7
LN_HALF = float(np.log(0.5))

_CONST_STATE = {}
bass_mod = None


def _c(nc, v):
    """[P,1] fp32 constant AP, DVE-memset once (keeps ACT single-wait)."""
    key = float(np.float32(v))
    consts = _CONST_STATE.setdefault(id(nc), {})
    if key not in consts:
        pool = _CONST_STATE[(id(nc), "pool")]
        from concourse import mybir
        t = pool.tile([P, 1], mybir.dt.float32, tag=f"c{len(consts)}", name=f"c{len(consts)}")
        nc.vector.memset(t, key)
        consts[key] = t
    return consts[key][:, 0:1]


def _pair_ap(tile, k0, stride_planes, n):
    """AP over n planes of `tile` ([P, 9, L] f16) starting at plane k0 with a
    plane-stride of `stride_planes` (may be negative)."""
    p0 = tile[:, k0, :]
    p1 = tile[:, k0 + 1, :] if k0 + 1 < 9 else tile[:, k0 - 1, :]
    do = p1.offset - p0.offset
    if k0 + 1 >= 9:
        do = -do
    return bass_mod.AP(tensor=p0.tensor, offset=p0.offset,
                       ap=[p0.ap[0], [do * stride_planes, n], p0.ap[1]])


def _bc(plane, k):
    """broadcast [P, L] plane across k planes -> [P, k, L]"""
    return bass_mod.AP(tensor=plane.tensor, offset=plane.offset,
                       ap=[plane.ap[0], [0, k], plane.ap[1]])


def _build_nc():
    global bass_mod
    import concourse.bass as bass
    import concourse.tile as tile
    from concourse import mybir
    bass_mod = bass

    f32 = mybir.dt.float32
    f16 = mybir.dt.float16
    Alu = mybir.AluOpType
    Act = mybir.ActivationFunctionType

    nc = bass.Bass()
    pred = nc.dram_tensor("pred", [ROWS_PER_CORE, 9], f32, kind="ExternalInput")
    targ = nc.dram_tensor("target", [ROWS_PER_CORE, 9], f32, kind="ExternalInput")
    out = nc.dram_tensor("partials", [P, 2 * NCHUNK], f32, kind="ExternalOutput")

    predv = pred.rearrange("(p n) c -> p n c", p=P)    # [128, 1024, 9]
    targv = targ.rearrange("(p n) c -> p n c", p=P)

    def mul(o, a, b):
        nc.vector.tensor_tensor(out=o, in0=a, in1=b, op=Alu.mult)

    def add(o, a, b):
        nc.vector.tensor_tensor(out=o, in0=a, in1=b, op=Alu.add)

    def sub(o, a, b):
        nc.vector.tensor_tensor(out=o, in0=a, in1=b, op=Alu.subtract)

    with tile.TileContext(nc) as tc:
        with (
            tc.tile_pool(name="raw", bufs=1) as rawp,
            tc.tile_pool(name="pl", bufs=1) as pl,
            tc.tile_pool(name="acc", bufs=1) as accp,
        ):
            acc = accp.tile([P, 2 * NCHUNK], f32, tag="acc")
            bias0 = accp.tile([P, 1], f32, tag="bias0")
            nc.vector.memset(bias0, 0.0)
            _CONST_STATE[(id(nc), "pool")] = accp

            def act(o, a, func, scale=1.0, bias=None, accum_out=None):
                if func == "Copy":
                    nc.scalar.activation(out=o, in_=a, func=Act.Copy,
                                         bias=0.0, scale=float(scale),
                                         accum_out=accum_out)
                else:
                    nc.scalar.activation(
                        out=o, in_=a, func=getattr(Act, func),
                        bias=bias0[:, 0:1] if bias is None else bias,
                        scale=float(scale), accum_out=accum_out)

            praw = rawp.tile([P, T * 9], f32, tag="praw")   # reused by chunks
            traw = rawp.tile([P, T * 9], f32, tag="traw")
            D = rawp.tile([P, 9, T], f16, tag="D")          # mse diff (shared)
            Z = [pl.tile([P, 9, L], f16, tag=f"Z{c}", name=f"Z{c}") for c in range(NCHUNK)]
            C = [pl.tile([P, 9, L], f16, tag=f"C{c}", name=f"C{c}") for c in range(NCHUNK)]
            W = [pl.tile([P, 9, L], f16, tag=f"W{c}", name=f"W{c}") for c in range(NCHUNK)]
            M3 = pl.tile([P, 3, L], f32, tag="M3")          # shared scratch
            dd = [pl.tile([P, L], f32, tag=f"d{c}", name=f"d{c}") for c in range(NCHUNK)]
            qq = [pl.tile([P, L], f32, tag=f"q{c}", name=f"q{c}") for c in range(NCHUNK)]
            bb = [pl.tile([P, L], f16, tag=f"b{c}", name=f"b{c}") for c in range(NCHUNK)]
            aa = [pl.tile([P, L], f16, tag=f"am{c}", name=f"am{c}") for c in range(NCHUNK)]
            sg = [pl.tile([P, L], f16, tag=f"sg{c}", name=f"sg{c}") for c in range(NCHUNK)]

            def load_and_deint(ch):
                # DMA raw chunk, then ACT copy-cast deinterleave into planes:
                # Z[ch][:, comp, 0:T] = pred rows, [T:L] = target rows
                nc.sync.dma_start(out=praw, in_=predv[:, ch*T:(ch+1)*T, :])
                nc.sync.dma_start(out=traw, in_=targv[:, ch*T:(ch+1)*T, :])
                for raw, half in ((praw, 0), (traw, 1)):
                    rv = raw.rearrange("p (n c) -> p n c", c=9)
                    xin = bass_mod.AP(tensor=rv.tensor, offset=rv.offset,
                                      ap=[rv.ap[0], rv.ap[2], rv.ap[1]])
                    act(Z[ch][:, :, half*T:(half+1)*T], xin, "Copy")

            def mse(ch):
                sub(D[:, :, :], Z[ch][:, :, 0:T], Z[ch][:, :, T:L])
                act(D[:, :, :], D[:, :, :], "Square",
                    accum_out=acc[:, ch:ch+1])

            def cof_det_act(ch, it):
                first = it == 0
                last = it == K_ITERS - 1
                z, c, w = Z[ch], C[ch], W[ch]
                if not first:
                    zf = z.rearrange("p c n -> p (c n)")
                    nc.vector.tensor_scalar(out=zf, in0=zf,
                                            scalar1=CLAMP_IT, scalar2=-CLAMP_IT,
                                            op0=Alu.min, op1=Alu.max)
                # cofactors: C[i,j] = z[i1,j1]z[i2,j2] - z[i1,j2]z[i2,j1]
                # pair j in {0,1} batched (regular plane strides), j=2 single
                for i in range(3):
                    i1, i2 = (i + 1) % 3, (i + 2) % 3
                    # pairs (i,0),(i,1):
                    A1 = _pair_ap(z, 3*i1 + 1, 1, 2)
                    A2 = _pair_ap(z, 3*i2 + 2, -2, 2)
                    A3 = _pair_ap(z, 3*i1 + 2, -2, 2)
                    A4 = _pair_ap(z, 3*i2 + 1, 1, 2)
                    o1 = _pair_ap(w, 3*i, 1, 2)
                    o2 = _pair_ap(c, 3*i, 1, 2)
                    mul(o1, A1, A2)
                    mul(o2, A3, A4)
                    sub(o2, o1, o2)
                    # single (i,2):
                    mul(w[:, 3*i+2, :], z[:, 3*i1, :], z[:, 3*i2+1, :])
                    mul(c[:, 3*i+2, :], z[:, 3*i1+1, :], z[:, 3*i2, :])
                    sub(c[:, 3*i+2, :], w[:, 3*i+2, :], c[:, 3*i+2, :])
                # det (fp32): d = sum_j z[0,j]*C[0,j]
                mul(M3[:, :, :], z[:, 0:3, :], c[:, 0:3, :])
                add(dd[ch], M3[:, 0, :], M3[:, 1, :])
                add(dd[ch], dd[ch], M3[:, 2, :])
                # ACT chain: a = sign*d^(-1/3), b = d^(-2/3)  (ln-domain)
                ofs = _c(nc, LN_HALF) if last else bias0[:, 0:1]
                act(qq[ch], dd[ch], "Square")
                act(qq[ch], qq[ch], "Ln", bias=_c(nc, EPS_D))
                act(bb[ch], qq[ch], "Exp", scale=-1.0/3.0, bias=ofs)
                act(aa[ch], qq[ch], "Exp", scale=-1.0/6.0, bias=ofs)
                if first:
                    act(sg[ch], dd[ch], "Sign")

            def update(ch, it):
                z, c, w = Z[ch], C[ch], W[ch]
                if it == 0:
                    mul(sg[ch], sg[ch], aa[ch])     # sg <- a16 (signed)
                a16 = sg[ch] if it == 0 else aa[ch]
                mul(w[:, :, :], z[:, :, :], _bc(a16, 9))
                mul(c[:, :, :], c[:, :, :], _bc(bb[ch], 9))
                zf = z.rearrange("p c n -> p (c n)")
                add(zf, w.rearrange("p c n -> p (c n)"),
                    c.rearrange("p c n -> p (c n)"))

            def rot(ch):
                z, w = Z[ch], W[ch]
                zf = z.rearrange("p c n -> p (c n)")
                nc.vector.tensor_scalar(out=zf, in0=zf, scalar1=8.0,
                                        scalar2=-8.0, op0=Alu.min, op1=Alu.max)
                sub(w[:, :, 0:T], z[:, :, 0:T], z[:, :, T:L])
                act(w[:, :, 0:T], w[:, :, 0:T], "Square",
                    accum_out=acc[:, NCHUNK+ch:NCHUNK+ch+1])

            # ---- software-pipelined schedule over the two chunks ----
            load_and_deint(0)
            mse(0)
            load_and_deint(1)   # praw reuse: DMA waits on chunk-0 deint (WAR)
            cof_det_act(0, 0)
            mse(1)
            cof_det_act(1, 0)
            update(0, 0)
            update(1, 0)
            for it in range(1, K_ITERS):
                cof_det_act(0, it)
                cof_det_act(1, it)
                update(0, it)
                update(1, it)
            rot(0)
            rot(1)
            nc.sync.dma_start(out=out[:, :], in_=acc)
    return nc


def _elide_implied_waits(nc):
    """Drop semaphore waits already implied by program order or transitively
    by earlier waits (vector-clock propagation).  Tile's per-instruction wait
    emission is not transitively minimal, and walrus can encode only one sync
    wait on Activation/DMA instructions (and ~4 on control instructions), so
    the redundant waits both break codegen and waste sequencer time.

    Model: each semaphore s carries a snapshot VC at every increment value;
    an engine's observed VC advances via its own instruction stream and via
    the snapshots of the waits it executes.  A wait (s >= v) is dropped iff
    the engine's observed VC already dominates it.  Unknown update modes
    disable elision for that semaphore (conservative).
    """
    join = lambda a, b: {k: max(a.get(k, 0), b.get(k, 0)) for k in set(a) | set(b)}
    sem_val = {}        # sem name -> current value
    sem_snap = {}       # sem name -> list of (value, VC) snapshots
    eng_vc = {}         # engine name -> observed VC
    unsafe = set()      # sems with non-increment updates
    n_drop = 0
    for f in nc.m.functions:
        for bb in f.blocks:
            for ins in bb.instructions:
                eng = str(ins.engine)
                vc = dict(eng_vc.get(eng, {}))
                si = ins.sync_info
                waits = list(si.on_wait) if si is not None and si.on_wait else []
                kept = []
                for w in waits:
                    s, v = w.ant_name, w.wait_value
                    if w.wait_mode != "sem-ge-imm" or s in unsafe:
                        kept.append(w)
                        continue
                    if vc.get(s, 0) >= v:
                        n_drop += 1
                        continue
                    if sem_val.get(s, 0) < v:
                        kept.append(w)
                        continue
                    kept.append(w)
                    snap = {}
                    for sv, svc in sem_snap.get(s, ()):
                        if sv <= v:
                            snap = svc
                        else:
                            break
                    vc = join(vc, snap)
                    vc[s] = max(vc.get(s, 0), v)
                if si is not None and len(kept) != len(waits):
                    si.on_wait = kept
                ups = si.on_update if si is not None and si.on_update else []
                for u in ups:
                    s = u.ant_name
                    if u.update_mode not in ("sem-inc", "sem-add-imm"):
                        unsafe.add(s)
                        continue
                    nv = sem_val.get(s, 0) + (u.update_value or 1)
                    sem_val[s] = nv
                    lst = sem_snap.setdefault(s, [])
                    prev = lst[-1][1] if lst else {}
                    lst.append((nv, join(prev, vc)))
                    if "DMA" not in s:
                        vc[s] = max(vc.get(s, 0), nv)
                eng_vc[eng] = vc
    return n_drop


_NC_CACHE = None


def kernel(pred: np.ndarray, target: np.ndarray) -> np.ndarray:
    global _NC_CACHE
    from concourse.bass_utils import run_bass_kernel_spmd

    pred = np.ascontiguousarray(np.asarray(pred, dtype=np.float32))
    target = np.ascontiguousarray(np.asarray(target, dtype=np.float32))
    assert pred.shape == (B, 9) and target.shape == (B, 9)

    if _NC_CACHE is None:
        _NC_CACHE = _build_nc()
        _elide_implied_waits(_NC_CACHE)
    nc = _NC_CACHE

    ps = pred.reshape(N_CORES, ROWS_PER_CORE, 9)
    ts = target.reshape(N_CORES, ROWS_PER_CORE, 9)
    in_maps = [{"pred": ps[i], "target": ts[i]} for i in range(N_CORES)]
    res = run_bass_kernel_spmd(nc, in_maps, core_ids=list(range(N_CORES)))
    globals()["_LAST_RESULT"] = res

    mse_sum = 0.0
    rot_sum = 0.0
    for r in res.results:
        part = np.asarray(r["partials"], dtype=np.float64)
        mse_sum += part[:, :NCHUNK].sum()
        rot_sum += part[:, NCHUNK:].sum()
    n = float(B * 9)
    return np.asarray(np.float32(mse_sum / n + 0.5 * (rot_sum / n)))


# revision 14
# speedup vs baseline: 1.9799x; 1.0810x over previous
"""CustomPoseLoss Trainium2 kernel.

loss = mean((pred-target)^2) + 0.5 * mean((R(pred)-R(target))^2)
where R(M) = sign(det M) * polar(M) for each 3x3 matrix (row of 9).

Implementation: det-scaled Newton iteration for the polar factor, with the
sign fix folded into the first iteration's scaling (R = polar(sign(det M)*M),
and the signed cube root sign(d)*|d|^{-1/3} handles it for free):

  Z_0 = M
  Z_{k+1} = a_k * Z_k + b_k * cof(Z_k),   a = d^{-1/3}, b = d^{-2/3}
  (d = det Z_k; the 0.5 Newton averaging is deferred into the scaling and
   applied via a ln(0.5) bias on the final iteration's exponentials)

All plane arithmetic is f16 unit-stride so DVE tensor_tensor runs in 2x mode;
det is accumulated in fp32 (avoids f16 inf -> NaN); iterates are clamped to
+-180 before each cofactor pass so every f16 product stays below 65504.
The transcendental chain (Square/Ln/Exp/Sign) runs on the Scalar engine in
parallel with the Vector engine's cofactor work of the other chunk
(two chunks are software-pipelined for exactly this overlap).

Sharding: pure data parallel over 8 cores; each core reduces its shard to
[128, 2*NCHUNK] partial sums (mse, rot), host combines in float64.
"""

import numpy as np

B = 1048576
N_CORES = 8
ROWS_PER_CORE = B // N_CORES          # 131072
P = 128
ROWS_PER_PART = ROWS_PER_CORE // P    # 1024
T = 512                               # rows per partition per chunk (per tensor)
NCHUNK = ROWS_PER_PART // T           # 2
L = 2 * T                             # plane width: [pred rows | target rows]
K_ITERS = 3
CLAMP_IT = 180.0
EPS_D = 1e-7
LN_HALF = float(np.log(0.5))

_CONST_STATE = {}
bass_mod = None


def _c(nc, v):
    """[P,1] fp32 constant AP, DVE-memset once (keeps ACT single-wait)."""
    key = float(np.float32(v))
    consts = _CONST_STATE.setdefault(id(nc), {})
    if key not in consts:
        pool = _CONST_STATE[(id(nc), "pool")]
        from concourse import mybir
        t = pool.tile([P, 1], mybir.dt.float32, tag=f"c{len(consts)}", name=f"c{len(consts)}")
        nc.vector.memset(t, key)
        consts[key] = t
    return consts[key][:, 0:1]


def _pair_ap(tile, k0, stride_planes, n):
    """AP over n planes of `tile` ([P, 9, L] f16) starting at plane k0 with a
    plane-stride of `stride_planes` (may be negative)."""
    p0 = tile[:, k0, :]
    p1 = tile[:, k0 + 1, :] if k0 + 1 < 9 else tile[:, k0 - 1, :]
    do = p1.offset - p0.offset
    if k0 + 1 >= 9:
        do = -do
    return bass_mod.AP(tensor=p0.tensor, offset=p0.offset,
                       ap=[p0.ap[0], [do * stride_planes, n], p0.ap[1]])


def _bc(plane, k):
    """broadcast [P, L] plane across k planes -> [P, k, L]"""
    return bass_mod.AP(tensor=plane.tensor, offset=plane.offset,
                       ap=[plane.ap[0], [0, k], plane.ap[1]])


def _build_nc():
    global bass_mod
    import concourse.bass as bass
    import concourse.tile as tile
    from concourse import mybir
    bass_mod = bass

    f32 = mybir.dt.float32
    f16 = mybir.dt.float16
    Alu = mybir.AluOpType
    Act = mybir.ActivationFunctionType

    nc = bass.Bass()
    pred = nc.dram_tensor("pred", [ROWS_PER_CORE, 9], f32, kind="ExternalInput")
    targ = nc.dram_tensor("target", [ROWS_PER_CORE, 9], f32, kind="ExternalInput")
    out = nc.dram_tensor("partials", [P, 2 * NCHUNK], f32, kind="ExternalOutput")

    predv = pred.rearrange("(p n) c -> p n c", p=P)    # [128, 1024, 9]
    targv = targ.rearrange("(p n) c -> p n c", p=P)

    def mul(o, a, b):
        nc.vector.tensor_tensor(out=o, in0=a, in1=b, op=Alu.mult)

    def add(o, a, b):
        nc.vector.tensor_tensor(out=o, in0=a, in1=b, op=Alu.add)

    def sub(o, a, b):
        nc.vector.tensor_tensor(out=o, in0=a, in1=b, op=Alu.subtract)

    with tile.TileContext(nc) as tc:
        with (
            tc.tile_pool(name="raw", bufs=1) as rawp,
            tc.tile_pool(name="pl", bufs=1) as pl,
            tc.tile_pool(name="acc", bufs=1) as accp,
        ):
            acc = accp.tile([P, 2 * NCHUNK], f32, tag="acc")
            bias0 = accp.tile([P, 1], f32, tag="bias0")
            nc.vector.memset(bias0, 0.0)
            _CONST_STATE[(id(nc), "pool")] = accp

            def act(o, a, func, scale=1.0, bias=None, accum_out=None):
                if func == "Copy":
                    nc.scalar.activation(out=o, in_=a, func=Act.Copy,
                                         bias=0.0, scale=float(scale),
                                         accum_out=accum_out)
                else:
                    nc.scalar.activation(
                        out=o, in_=a, func=getattr(Act, func),
                        bias=bias0[:, 0:1] if bias is None else bias,
                        scale=float(scale), accum_out=accum_out)

            praw = [rawp.tile([P, T * 9], f32, tag=f"praw{c}", name=f"praw{c}")
                    for c in range(NCHUNK)]
            traw = [rawp.tile([P, T * 9], f32, tag=f"traw{c}", name=f"traw{c}")
                    for c in range(NCHUNK)]
            D = rawp.tile([P, 9, T], f16, tag="D")          # mse diff (shared)
            Z = [pl.tile([P, 9, L], f16, tag=f"Z{c}", name=f"Z{c}") for c in range(NCHUNK)]
            C = [pl.tile([P, 9, L], f16, tag=f"C{c}", name=f"C{c}") for c in range(NCHUNK)]
            W = pl.tile([P, 9, L], f16, tag="W")            # shared scratch
            M3 = pl.tile([P, 3, L], f32, tag="M3")          # shared scratch
            dd = [pl.tile([P, L], f32, tag=f"d{c}", name=f"d{c}") for c in range(NCHUNK)]
            bb = [pl.tile([P, L], f16, tag=f"b{c}", name=f"b{c}") for c in range(NCHUNK)]
            aa = [pl.tile([P, L], f16, tag=f"am{c}", name=f"am{c}") for c in range(NCHUNK)]
            sg = [pl.tile([P, L], f16, tag=f"sg{c}", name=f"sg{c}") for c in range(NCHUNK)]

            def load(ch, pieces=1):
                # DMA raw chunk in row-pieces so deint can pipeline behind DMA
                for pc in range(pieces):
                    r0, r1 = ch*T + pc*T//pieces, ch*T + (pc+1)*T//pieces
                    s0, s1 = pc*T//pieces * 9, (pc+1)*T//pieces * 9
                    nc.sync.dma_start(out=praw[ch][:, s0:s1],
                                      in_=predv[:, r0:r1, :])
                    nc.sync.dma_start(out=traw[ch][:, s0:s1],
                                      in_=targv[:, r0:r1, :])

            def deint(ch, pieces=1):
                # ACT copy-cast deinterleave into planes (piece-major, to
                # match DMA landing order):
                # Z[ch][:, comp, 0:T] = pred rows, [T:L] = target rows
                for pc in range(pieces):
                    n0, n1 = pc*T//pieces, (pc+1)*T//pieces
                    for raw, half in ((praw[ch], 0), (traw[ch], 1)):
                        rv = raw.rearrange("p (n c) -> p n c", c=9)
                        xin = bass_mod.AP(tensor=rv.tensor, offset=rv.offset,
                                          ap=[rv.ap[0], rv.ap[2], rv.ap[1]])
                        xi = bass_mod.AP(tensor=xin.tensor,
                                         offset=xin.offset + n0*xin.ap[2][0],
                                         ap=[xin.ap[0], xin.ap[1],
                                             [xin.ap[2][0], n1 - n0]])
                        act(Z[ch][:, :, half*T+n0:half*T+n1], xi, "Copy")

            def mse(ch):
                sub(D[:, :, :], Z[ch][:, :, 0:T], Z[ch][:, :, T:L])
                act(D[:, :, :], D[:, :, :], "Square",
                    accum_out=acc[:, ch:ch+1])

            def cof_det_act(ch, it):
                first = it == 0
                last = it == K_ITERS - 1
                z, c, w = Z[ch], C[ch], W
                if not first:
                    zf = z.rearrange("p c n -> p (c n)")
                    nc.vector.tensor_scalar(out=zf, in0=zf,
                                            scalar1=CLAMP_IT, scalar2=-CLAMP_IT,
                                            op0=Alu.min, op1=Alu.max)
                # cofactors: C[i,j] = z[i1,j1]z[i2,j2] - z[i1,j2]z[i2,j1]
                # pair j in {0,1} batched (regular plane strides), j=2 single
                for i in range(3):
                    i1, i2 = (i + 1) % 3, (i + 2) % 3
                    # pairs (i,0),(i,1):
                    A1 = _pair_ap(z, 3*i1 + 1, 1, 2)
                    A2 = _pair_ap(z, 3*i2 + 2, -2, 2)
                    A3 = _pair_ap(z, 3*i1 + 2, -2, 2)
                    A4 = _pair_ap(z, 3*i2 + 1, 1, 2)
                    o1 = _pair_ap(w, 3*i, 1, 2)
                    o2 = _pair_ap(c, 3*i, 1, 2)
                    mul(o1, A1, A2)
                    mul(o2, A3, A4)
                    sub(o2, o1, o2)
                    # single (i,2):
                    mul(w[:, 3*i+2, :], z[:, 3*i1, :], z[:, 3*i2+1, :])
                    mul(c[:, 3*i+2, :], z[:, 3*i1+1, :], z[:, 3*i2, :])
                    sub(c[:, 3*i+2, :], w[:, 3*i+2, :], c[:, 3*i+2, :])
                # det (fp32): d = sum_j z[0,j]*C[0,j]
                mul(M3[:, :, :], z[:, 0:3, :], c[:, 0:3, :])
                add(dd[ch], M3[:, 0, :], M3[:, 1, :])
                add(dd[ch], dd[ch], M3[:, 2, :])
                # ACT chain: a = sign(d)*|d|^(-1/3) [, b = |d|^(-2/3) on last]
                # Sign first, then Square/Ln run in place on dd.
                ofs = _c(nc, LN_HALF) if last else bias0[:, 0:1]
                act(sg[ch], dd[ch], "Sign")
                act(dd[ch], dd[ch], "Square")
                act(dd[ch], dd[ch], "Ln", bias=_c(nc, EPS_D))
                if last:
                    act(bb[ch], dd[ch], "Exp", scale=-1.0/3.0, bias=ofs)
                act(aa[ch], dd[ch], "Exp", scale=-1.0/6.0, bias=ofs)

            def update(ch, it):
                last = it == K_ITERS - 1
                z, c = Z[ch], C[ch]
                mul(sg[ch], sg[ch], aa[ch])         # sg <- a16 (signed)
                zf = z.rearrange("p c n -> p (c n)")
                cf = c.rearrange("p c n -> p (c n)")
                if not last:
                    # drift update: Z <- Z + a*C (scale absorbed by next det)
                    mul(c[:, :, :], c[:, :, :], _bc(sg[ch], 9))
                    add(zf, zf, cf)
                else:
                    mul(W[:, :, :], z[:, :, :], _bc(sg[ch], 9))
                    mul(c[:, :, :], c[:, :, :], _bc(bb[ch], 9))
                    add(zf, W.rearrange("p c n -> p (c n)"), cf)

            def rot(ch, buf):
                z = Z[ch]
                zf = z.rearrange("p c n -> p (c n)")
                nc.vector.tensor_scalar(out=zf, in0=zf, scalar1=8.0,
                                        scalar2=-8.0, op0=Alu.min, op1=Alu.max)
                sub(buf[:, :, 0:T], z[:, :, 0:T], z[:, :, T:L])
                act(buf[:, :, 0:T], buf[:, :, 0:T], "Square",
                    accum_out=acc[:, NCHUNK+ch:NCHUNK+ch+1])

            # ---- software-pipelined schedule over the two chunks ----
            load(0, pieces=2)
            load(1, pieces=2)
            deint(0, pieces=2)
            mse(0)
            deint(1, pieces=2)
            cof_det_act(0, 0)
            mse(1)
            cof_det_act(1, 0)
            update(0, 0)
            update(1, 0)
            for it in range(1, K_ITERS):
                cof_det_act(0, it)
                cof_det_act(1, it)
                update(0, it)
                update(1, it)
            rot(0, D)
            rot(1, W)
            nc.sync.dma_start(out=out[:, :], in_=acc)
    return nc


def _elide_implied_waits(nc):
    """Drop semaphore waits already implied by program order or transitively
    by earlier waits (vector-clock propagation).  Tile's per-instruction wait
    emission is not transitively minimal, and walrus can encode only one sync
    wait on Activation/DMA instructions (and ~4 on control instructions), so
    the redundant waits both break codegen and waste sequencer time.

    Model: each semaphore s carries a snapshot VC at every increment value;
    an engine's observed VC advances via its own instruction stream and via
    the snapshots of the waits it executes.  A wait (s >= v) is dropped iff
    the engine's observed VC already dominates it.  Unknown update modes
    disable elision for that semaphore (conservative).
    """
    join = lambda a, b: {k: max(a.get(k, 0), b.get(k, 0)) for k in set(a) | set(b)}
    sem_val = {}        # sem name -> current value
    sem_snap = {}       # sem name -> list of (value, VC) snapshots
    eng_vc = {}         # engine name -> observed VC
    unsafe = set()      # sems with non-increment updates
    n_drop = 0
    for f in nc.m.functions:
        for bb in f.blocks:
            for ins in bb.instructions:
                eng = str(ins.engine)
                vc = dict(eng_vc.get(eng, {}))
                si = ins.sync_info
                waits = list(si.on_wait) if si is not None and si.on_wait else []
                kept = []
                for w in waits:
                    s, v = w.ant_name, w.wait_value
                    if w.wait_mode != "sem-ge-imm" or s in unsafe:
                        kept.append(w)
                        continue
                    if vc.get(s, 0) >= v:
                        n_drop += 1
                        continue
                    if sem_val.get(s, 0) < v:
                        kept.append(w)
                        continue
                    kept.append(w)
                    snap = {}
                    for sv, svc in sem_snap.get(s, ()):
                        if sv <= v:
                            snap = svc
                        else:
                            break
                    vc = join(vc, snap)
                    vc[s] = max(vc.get(s, 0), v)
                if si is not None and len(kept) != len(waits):
                    si.on_wait = kept
                ups = si.on_update if si is not None and si.on_update else []
                for u in ups:
                    s = u.ant_name
                    if u.update_mode not in ("sem-inc", "sem-add-imm"):
                        unsafe.add(s)
                        continue
                    nv = sem_val.get(s, 0) + (u.update_value or 1)
                    sem_val[s] = nv
                    lst = sem_snap.setdefault(s, [])
                    prev = lst[-1][1] if lst else {}
                    lst.append((nv, join(prev, vc)))
                    if "DMA" not in s:
                        vc[s] = max(vc.get(s, 0), nv)
                eng_vc[eng] = vc
    return n_drop


_NC_CACHE = None


def kernel(pred: np.ndarray, target: np.ndarray) -> np.ndarray:
    global _NC_CACHE
    from concourse.bass_utils import run_bass_kernel_spmd

    pred = np.ascontiguousarray(np.asarray(pred, dtype=np.float32))
    target = np.ascontiguousarray(np.asarray(target, dtype=np.float32))
    assert pred.shape == (B, 9) and target.shape == (B, 9)

    if _NC_CACHE is None:
        _NC_CACHE = _build_nc()
        _elide_implied_waits(_NC_CACHE)
    nc = _NC_CACHE

    ps = pred.reshape(N_CORES, ROWS_PER_CORE, 9)
    ts = target.reshape(N_CORES, ROWS_PER_CORE, 9)
    in_maps = [{"pred": ps[i], "target": ts[i]} for i in range(N_CORES)]
    res = run_bass_kernel_spmd(nc, in_maps, core_ids=list(range(N_CORES)))
    globals()["_LAST_RESULT"] = res

    mse_sum = 0.0
    rot_sum = 0.0
    for r in res.results:
        part = np.asarray(r["partials"], dtype=np.float64)
        mse_sum += part[:, :NCHUNK].sum()
        rot_sum += part[:, NCHUNK:].sum()
    n = float(B * 9)
    return np.asarray(np.float32(mse_sum / n + 0.5 * (rot_sum / n)))


# revision 18
# speedup vs baseline: 2.0441x; 1.0324x over previous
"""CustomPoseLoss Trainium2 kernel.

loss = mean((pred-target)^2) + 0.5 * mean((R(pred)-R(target))^2)
where R(M) = sign(det M) * polar(M) for each 3x3 matrix (row of 9).

Implementation: det-scaled Newton iteration for the polar factor, with the
sign fix folded into the first iteration's scaling (R = polar(sign(det M)*M),
and the signed cube root sign(d)*|d|^{-1/3} handles it for free):

  Z_0 = M
  Z_{k+1} = a_k * Z_k + b_k * cof(Z_k),   a = d^{-1/3}, b = d^{-2/3}
  (d = det Z_k; the 0.5 Newton averaging is deferred into the scaling and
   applied via a ln(0.5) bias on the final iteration's exponentials)

All plane arithmetic is f16 unit-stride so DVE tensor_tensor runs in 2x mode;
det is accumulated in fp32 (avoids f16 inf -> NaN); iterates are clamped to
+-180 before each cofactor pass so every f16 product stays below 65504.
The transcendental chain (Square/Ln/Exp/Sign) runs on the Scalar engine in
parallel with the Vector engine's cofactor work of the other chunk
(two chunks are software-pipelined for exactly this overlap).

Sharding: pure data parallel over 8 cores; each core reduces its shard to
[128, 2*NCHUNK] partial sums (mse, rot), host combines in float64.
"""

import numpy as np

B = 1048576
N_CORES = 8
ROWS_PER_CORE = B // N_CORES          # 131072
P = 128
ROWS_PER_PART = ROWS_PER_CORE // P    # 1024
T = 512                               # rows per partition per chunk (per tensor)
NCHUNK = ROWS_PER_PART // T           # 2
L = 2 * T                             # plane width: [pred rows | target rows]
K_ITERS = 3
CLAMP_IT = 180.0
EPS_D = 1e-7
LN_HALF = float(np.log(0.5))

_CONST_STATE = {}
bass_mod = None


def _c(nc, v):
    """[P,1] fp32 constant AP, DVE-memset once (keeps ACT single-wait)."""
    key = float(np.float32(v))
    consts = _CONST_STATE.setdefault(id(nc), {})
    if key not in consts:
        pool = _CONST_STATE[(id(nc), "pool")]
        from concourse import mybir
        t = pool.tile([P, 1], mybir.dt.float32, tag=f"c{len(consts)}", name=f"c{len(consts)}")
        nc.vector.memset(t, key)
        consts[key] = t
    return consts[key][:, 0:1]


def _pair_ap(tile, k0, stride_planes, n):
    """AP over n planes of `tile` ([P, 9, L] f16) starting at plane k0 with a
    plane-stride of `stride_planes` (may be negative)."""
    p0 = tile[:, k0, :]
    p1 = tile[:, k0 + 1, :] if k0 + 1 < 9 else tile[:, k0 - 1, :]
    do = p1.offset - p0.offset
    if k0 + 1 >= 9:
        do = -do
    return bass_mod.AP(tensor=p0.tensor, offset=p0.offset,
                       ap=[p0.ap[0], [do * stride_planes, n], p0.ap[1]])


def _bc(plane, k):
    """broadcast [P, L] plane across k planes -> [P, k, L]"""
    return bass_mod.AP(tensor=plane.tensor, offset=plane.offset,
                       ap=[plane.ap[0], [0, k], plane.ap[1]])


def _build_nc():
    global bass_mod
    import concourse.bass as bass
    import concourse.tile as tile
    from concourse import mybir
    bass_mod = bass

    f32 = mybir.dt.float32
    f16 = mybir.dt.float16
    Alu = mybir.AluOpType
    Act = mybir.ActivationFunctionType

    nc = bass.Bass()
    pred = nc.dram_tensor("pred", [ROWS_PER_CORE, 9], f32, kind="ExternalInput")
    targ = nc.dram_tensor("target", [ROWS_PER_CORE, 9], f32, kind="ExternalInput")
    out = nc.dram_tensor("partials", [P, 2 * NCHUNK], f32, kind="ExternalOutput")

    predv = pred.rearrange("(p n) c -> p n c", p=P)    # [128, 1024, 9]
    targv = targ.rearrange("(p n) c -> p n c", p=P)

    def mul(o, a, b):
        nc.vector.tensor_tensor(out=o, in0=a, in1=b, op=Alu.mult)

    def add(o, a, b):
        nc.vector.tensor_tensor(out=o, in0=a, in1=b, op=Alu.add)

    def sub(o, a, b):
        nc.vector.tensor_tensor(out=o, in0=a, in1=b, op=Alu.subtract)

    with tile.TileContext(nc) as tc:
        with (
            tc.tile_pool(name="raw", bufs=1) as rawp,
            tc.tile_pool(name="pl", bufs=1) as pl,
            tc.tile_pool(name="acc", bufs=1) as accp,
        ):
            acc = accp.tile([P, 2 * NCHUNK], f32, tag="acc")
            bias0 = accp.tile([P, 1], f32, tag="bias0")
            nc.vector.memset(bias0, 0.0)
            _CONST_STATE[(id(nc), "pool")] = accp

            def act(o, a, func, scale=1.0, bias=None, accum_out=None):
                if func == "Copy":
                    nc.scalar.activation(out=o, in_=a, func=Act.Copy,
                                         bias=0.0, scale=float(scale),
                                         accum_out=accum_out)
                else:
                    nc.scalar.activation(
                        out=o, in_=a, func=getattr(Act, func),
                        bias=bias0[:, 0:1] if bias is None else bias,
                        scale=float(scale), accum_out=accum_out)

            NP_ = 2   # DMA pieces per tensor-chunk (separate tiles so each
            TP = T // NP_   # deint copy waits only on its own piece's DMA)
            praw = [[rawp.tile([P, TP * 9], f32, tag=f"praw{c}_{j}",
                               name=f"praw{c}_{j}") for j in range(NP_)]
                    for c in range(NCHUNK)]
            traw = [[rawp.tile([P, TP * 9], f32, tag=f"traw{c}_{j}",
                               name=f"traw{c}_{j}") for j in range(NP_)]
                    for c in range(NCHUNK)]
            D = rawp.tile([P, 9, T], f16, tag="D")          # mse diff (shared)
            Z = [pl.tile([P, 9, L], f16, tag=f"Z{c}", name=f"Z{c}") for c in range(NCHUNK)]
            C = [pl.tile([P, 9, L], f16, tag=f"C{c}", name=f"C{c}") for c in range(NCHUNK)]
            W = pl.tile([P, 9, L], f16, tag="W")            # shared scratch
            M3 = pl.tile([P, 3, L], f32, tag="M3")          # shared scratch
            dd = [pl.tile([P, L], f32, tag=f"d{c}", name=f"d{c}") for c in range(NCHUNK)]
            bb = [pl.tile([P, L], f16, tag=f"b{c}", name=f"b{c}") for c in range(NCHUNK)]
            aa = [pl.tile([P, L], f16, tag=f"am{c}", name=f"am{c}") for c in range(NCHUNK)]
            sg = [pl.tile([P, L], f16, tag=f"sg{c}", name=f"sg{c}") for c in range(NCHUNK)]

            def load(ch):
                # DMA raw chunk in row-pieces; nc.sync DMAs run FIFO in
                # emission order at full BW, so chunk-0 pieces land first.
                for pc in range(NP_):
                    r0, r1 = ch*T + pc*TP, ch*T + (pc+1)*TP
                    nc.sync.dma_start(out=praw[ch][pc], in_=predv[:, r0:r1, :])
                    nc.sync.dma_start(out=traw[ch][pc], in_=targv[:, r0:r1, :])

            def deint(ch, dve_half=False):
                # copy-cast deinterleave into planes (piece-major, matching
                # DMA landing order): Z[:, comp, 0:T]=pred, [T:L]=target.
                # dve_half: route target-tensor copies to the (startup-idle)
                # Vector engine instead of ACT.
                for pc in range(NP_):
                    n0 = pc * TP
                    for raws, half in ((praw[ch], 0), (traw[ch], 1)):
                        rv = raws[pc].rearrange("p (n c) -> p n c", c=9)
                        xi = bass_mod.AP(tensor=rv.tensor, offset=rv.offset,
                                         ap=[rv.ap[0], rv.ap[2], rv.ap[1]])
                        o = Z[ch][:, :, half*T+n0:half*T+n0+TP]
                        if dve_half and half == 1:
                            nc.vector.tensor_copy(out=o, in_=xi)
                        else:
                            act(o, xi, "Copy")

            def mse(ch):
                sub(D[:, :, :], Z[ch][:, :, 0:T], Z[ch][:, :, T:L])
                act(D[:, :, :], D[:, :, :], "Square",
                    accum_out=acc[:, ch:ch+1])

            def cof_det_act(ch, it):
                first = it == 0
                last = it == K_ITERS - 1
                z, c, w = Z[ch], C[ch], W
                if not first:
                    zf = z.rearrange("p c n -> p (c n)")
                    nc.vector.tensor_scalar(out=zf, in0=zf,
                                            scalar1=CLAMP_IT, scalar2=-CLAMP_IT,
                                            op0=Alu.min, op1=Alu.max)
                # cofactors: C[i,j] = z[i1,j1]z[i2,j2] - z[i1,j2]z[i2,j1]
                # pair j in {0,1} batched (regular plane strides), j=2 single
                for i in range(3):
                    i1, i2 = (i + 1) % 3, (i + 2) % 3
                    # pairs (i,0),(i,1):
                    A1 = _pair_ap(z, 3*i1 + 1, 1, 2)
                    A2 = _pair_ap(z, 3*i2 + 2, -2, 2)
                    A3 = _pair_ap(z, 3*i1 + 2, -2, 2)
                    A4 = _pair_ap(z, 3*i2 + 1, 1, 2)
                    o1 = _pair_ap(w, 3*i, 1, 2)
                    o2 = _pair_ap(c, 3*i, 1, 2)
                    mul(o1, A1, A2)
                    mul(o2, A3, A4)
                    sub(o2, o1, o2)
                # singles (i,2): rows 0,1 pair cross-row (stride 3 / -6),
                # row 2 alone
                A1 = _pair_ap(z, 3, 3, 2)    # z[1,0], z[2,0]
                A2 = _pair_ap(z, 7, -6, 2)   # z[2,1], z[0,1]
                A3 = _pair_ap(z, 4, 3, 2)    # z[1,1], z[2,1]
                A4 = _pair_ap(z, 6, -6, 2)   # z[2,0], z[0,0]
                o1 = _pair_ap(w, 2, 3, 2)
                o2 = _pair_ap(c, 2, 3, 2)
                mul(o1, A1, A2)
                mul(o2, A3, A4)
                sub(o2, o1, o2)
                mul(w[:, 8, :], z[:, 0, :], z[:, 4, :])
                mul(c[:, 8, :], z[:, 1, :], z[:, 3, :])
                sub(c[:, 8, :], w[:, 8, :], c[:, 8, :])
                # det (fp32): d = sum_j z[0,j]*C[0,j]
                mul(M3[:, :, :], z[:, 0:3, :], c[:, 0:3, :])
                add(dd[ch], M3[:, 0, :], M3[:, 1, :])
                add(dd[ch], dd[ch], M3[:, 2, :])
                # ACT chain: a = sign(d)*|d|^(-1/3) [, b = |d|^(-2/3) on last]
                # Sign first, then Square/Ln run in place on dd.
                ofs = _c(nc, LN_HALF) if last else bias0[:, 0:1]
                act(sg[ch], dd[ch], "Sign")
                act(dd[ch], dd[ch], "Square")
                act(dd[ch], dd[ch], "Ln", bias=_c(nc, EPS_D))
                if last:
                    act(bb[ch], dd[ch], "Exp", scale=-1.0/3.0, bias=ofs)
                act(aa[ch], dd[ch], "Exp", scale=-1.0/6.0, bias=ofs)

            def update(ch, it):
                last = it == K_ITERS - 1
                z, c = Z[ch], C[ch]
                mul(sg[ch], sg[ch], aa[ch])         # sg <- a16 (signed)
                zf = z.rearrange("p c n -> p (c n)")
                cf = c.rearrange("p c n -> p (c n)")
                if not last:
                    # drift update: Z <- Z + a*C (scale absorbed by next det)
                    mul(c[:, :, :], c[:, :, :], _bc(sg[ch], 9))
                    add(zf, zf, cf)
                else:
                    mul(W[:, :, :], z[:, :, :], _bc(sg[ch], 9))
                    mul(c[:, :, :], c[:, :, :], _bc(bb[ch], 9))
                    add(zf, W.rearrange("p c n -> p (c n)"), cf)

            def rot(ch, buf):
                z = Z[ch]
                zf = z.rearrange("p c n -> p (c n)")
                nc.vector.tensor_scalar(out=zf, in0=zf, scalar1=8.0,
                                        scalar2=-8.0, op0=Alu.min, op1=Alu.max)
                sub(buf[:, :, 0:T], z[:, :, 0:T], z[:, :, T:L])
                act(buf[:, :, 0:T], buf[:, :, 0:T], "Square",
                    accum_out=acc[:, NCHUNK+ch:NCHUNK+ch+1])

            # ---- software-pipelined schedule over the two chunks ----
            load(0)
            load(1)
            deint(0, dve_half=True)
            mse(0)
            deint(1)
            cof_det_act(0, 0)
            mse(1)
            cof_det_act(1, 0)
            update(0, 0)
            update(1, 0)
            for it in range(1, K_ITERS):
                cof_det_act(0, it)
                cof_det_act(1, it)
                update(0, it)
                update(1, it)
            rot(0, D)
            rot(1, W)
            nc.sync.dma_start(out=out[:, :], in_=acc)
    return nc


def _elide_implied_waits(nc):
    """Drop semaphore waits already implied by program order or transitively
    by earlier waits (vector-clock propagation).  Tile's per-instruction wait
    emission is not transitively minimal, and walrus can encode only one sync
    wait on Activation/DMA instructions (and ~4 on control instructions), so
    the redundant waits both break codegen and waste sequencer time.

    Model: each semaphore s carries a snapshot VC at every increment value;
    an engine's observed VC advances via its own instruction stream and via
    the snapshots of the waits it executes.  A wait (s >= v) is dropped iff
    the engine's observed VC already dominates it.  Unknown update modes
    disable elision for that semaphore (conservative).
    """
    join = lambda a, b: {k: max(a.get(k, 0), b.get(k, 0)) for k in set(a) | set(b)}
    sem_val = {}        # sem name -> current value
    sem_snap = {}       # sem name -> list of (value, VC) snapshots
    eng_vc = {}         # engine name -> observed VC
    unsafe = set()      # sems with non-increment updates
    n_drop = 0
    for f in nc.m.functions:
        for bb in f.blocks:
            for ins in bb.instructions:
                eng = str(ins.engine)
                vc = dict(eng_vc.get(eng, {}))
                si = ins.sync_info
                waits = list(si.on_wait) if si is not None and si.on_wait else []
                kept = []
                for w in waits:
                    s, v = w.ant_name, w.wait_value
                    if w.wait_mode != "sem-ge-imm" or s in unsafe:
                        kept.append(w)
                        continue
                    if vc.get(s, 0) >= v:
                        n_drop += 1
                        continue
                    if sem_val.get(s, 0) < v:
                        kept.append(w)
                        continue
                    kept.append(w)
                    snap = {}
                    for sv, svc in sem_snap.get(s, ()):
                        if sv <= v:
                            snap = svc
                        else:
                            break
                    vc = join(vc, snap)
                    vc[s] = max(vc.get(s, 0), v)
                if si is not None and len(kept) != len(waits):
                    si.on_wait = kept
                ups = si.on_update if si is not None and si.on_update else []
                for u in ups:
                    s = u.ant_name
                    if u.update_mode not in ("sem-inc", "sem-add-imm"):
                        unsafe.add(s)
                        continue
                    nv = sem_val.get(s, 0) + (u.update_value or 1)
                    sem_val[s] = nv
                    lst = sem_snap.setdefault(s, [])
                    prev = lst[-1][1] if lst else {}
                    lst.append((nv, join(prev, vc)))
                    if "DMA" not in s:
                        vc[s] = max(vc.get(s, 0), nv)
                eng_vc[eng] = vc
    return n_drop


_NC_CACHE = None


def kernel(pred: np.ndarray, target: np.ndarray) -> np.ndarray:
    global _NC_CACHE
    from concourse.bass_utils import run_bass_kernel_spmd

    pred = np.ascontiguousarray(np.asarray(pred, dtype=np.float32))
    target = np.ascontiguousarray(np.asarray(target, dtype=np.float32))
    assert pred.shape == (B, 9) and target.shape == (B, 9)

    if _NC_CACHE is None:
        _NC_CACHE = _build_nc()
        _elide_implied_waits(_NC_CACHE)
    nc = _NC_CACHE

    ps = pred.reshape(N_CORES, ROWS_PER_CORE, 9)
    ts = target.reshape(N_CORES, ROWS_PER_CORE, 9)
    in_maps = [{"pred": ps[i], "target": ts[i]} for i in range(N_CORES)]
    res = run_bass_kernel_spmd(nc, in_maps, core_ids=list(range(N_CORES)))
    globals()["_LAST_RESULT"] = res

    mse_sum = 0.0
    rot_sum = 0.0
    for r in res.results:
        part = np.asarray(r["partials"], dtype=np.float64)
        mse_sum += part[:, :NCHUNK].sum()
        rot_sum += part[:, NCHUNK:].sum()
    n = float(B * 9)
    return np.asarray(np.float32(mse_sum / n + 0.5 * (rot_sum / n)))


# revision 19
# speedup vs baseline: 2.0541x; 1.0049x over previous
"""CustomPoseLoss Trainium2 kernel.

loss = mean((pred-target)^2) + 0.5 * mean((R(pred)-R(target))^2)
where R(M) = sign(det M) * polar(M) for each 3x3 matrix (row of 9).

Implementation: det-scaled Newton iteration for the polar factor, with the
sign fix folded into the first iteration's scaling (R = polar(sign(det M)*M),
and the signed cube root sign(d)*|d|^{-1/3} handles it for free):

  Z_0 = M
  Z_{k+1} = a_k * Z_k + b_k * cof(Z_k),   a = d^{-1/3}, b = d^{-2/3}
  (d = det Z_k; the 0.5 Newton averaging is deferred into the scaling and
   applied via a ln(0.5) bias on the final iteration's exponentials)

All plane arithmetic is f16 unit-stride so DVE tensor_tensor runs in 2x mode;
det is accumulated in fp32 (avoids f16 inf -> NaN); iterates are clamped to
+-180 before each cofactor pass so every f16 product stays below 65504.
The transcendental chain (Square/Ln/Exp/Sign) runs on the Scalar engine in
parallel with the Vector engine's cofactor work of the other chunk
(two chunks are software-pipelined for exactly this overlap).

Sharding: pure data parallel over 8 cores; each core reduces its shard to
[128, 2*NCHUNK] partial sums (mse, rot), host combines in float64.
"""

import numpy as np

B = 1048576
N_CORES = 8
ROWS_PER_CORE = B // N_CORES          # 131072
P = 128
ROWS_PER_PART = ROWS_PER_CORE // P    # 1024
T = 512                               # rows per partition per chunk (per tensor)
NCHUNK = ROWS_PER_PART // T           # 2
L = 2 * T                             # plane width: [pred rows | target rows]
K_ITERS = 3
CLAMP_IT = 180.0
EPS_D = 1e-7
LN_HALF = float(np.log(0.5))

_CONST_STATE = {}
bass_mod = None


def _c(nc, v):
    """[P,1] fp32 constant AP, DVE-memset once (keeps ACT single-wait)."""
    key = float(np.float32(v))
    consts = _CONST_STATE.setdefault(id(nc), {})
    if key not in consts:
        pool = _CONST_STATE[(id(nc), "pool")]
        from concourse import mybir
        t = pool.tile([P, 1], mybir.dt.float32, tag=f"c{len(consts)}", name=f"c{len(consts)}")
        nc.vector.memset(t, key)
        consts[key] = t
    return consts[key][:, 0:1]


def _pair_ap(tile, k0, stride_planes, n):
    """AP over n planes of `tile` ([P, 9, L] f16) starting at plane k0 with a
    plane-stride of `stride_planes` (may be negative)."""
    p0 = tile[:, k0, :]
    p1 = tile[:, k0 + 1, :] if k0 + 1 < 9 else tile[:, k0 - 1, :]
    do = p1.offset - p0.offset
    if k0 + 1 >= 9:
        do = -do
    return bass_mod.AP(tensor=p0.tensor, offset=p0.offset,
                       ap=[p0.ap[0], [do * stride_planes, n], p0.ap[1]])


def _bc(plane, k):
    """broadcast [P, L] plane across k planes -> [P, k, L]"""
    return bass_mod.AP(tensor=plane.tensor, offset=plane.offset,
                       ap=[plane.ap[0], [0, k], plane.ap[1]])


def _build_nc():
    global bass_mod
    import concourse.bass as bass
    import concourse.tile as tile
    from concourse import mybir
    bass_mod = bass

    f32 = mybir.dt.float32
    f16 = mybir.dt.float16
    Alu = mybir.AluOpType
    Act = mybir.ActivationFunctionType

    nc = bass.Bass()
    pred = nc.dram_tensor("pred", [ROWS_PER_CORE, 9], f32, kind="ExternalInput")
    targ = nc.dram_tensor("target", [ROWS_PER_CORE, 9], f32, kind="ExternalInput")
    out = nc.dram_tensor("partials", [P, 2 + NCHUNK + 1], f32,
                         kind="ExternalOutput")

    predv = pred.rearrange("(p n) c -> p n c", p=P)    # [128, 1024, 9]
    targv = targ.rearrange("(p n) c -> p n c", p=P)

    def mul(o, a, b):
        nc.vector.tensor_tensor(out=o, in0=a, in1=b, op=Alu.mult)

    def add(o, a, b):
        nc.vector.tensor_tensor(out=o, in0=a, in1=b, op=Alu.add)

    def sub(o, a, b):
        nc.vector.tensor_tensor(out=o, in0=a, in1=b, op=Alu.subtract)

    with tile.TileContext(nc) as tc:
        with (
            tc.tile_pool(name="raw", bufs=1) as rawp,
            tc.tile_pool(name="pl", bufs=1) as pl,
            tc.tile_pool(name="acc", bufs=1) as accp,
        ):
            acc = accp.tile([P, 2 + NCHUNK + 1], f32, tag="acc")
            bias0 = accp.tile([P, 1], f32, tag="bias0")
            nc.vector.memset(bias0, 0.0)
            _CONST_STATE[(id(nc), "pool")] = accp

            def act(o, a, func, scale=1.0, bias=None, accum_out=None):
                if func == "Copy":
                    nc.scalar.activation(out=o, in_=a, func=Act.Copy,
                                         bias=0.0, scale=float(scale),
                                         accum_out=accum_out)
                else:
                    nc.scalar.activation(
                        out=o, in_=a, func=getattr(Act, func),
                        bias=bias0[:, 0:1] if bias is None else bias,
                        scale=float(scale), accum_out=accum_out)

            NP_ = 4   # DMA pieces per tensor-chunk (separate tiles so each
            TP = T // NP_   # deint copy waits only on its own piece's DMA)
            praw = [[rawp.tile([P, TP * 9], f32, tag=f"praw{c}_{j}",
                               name=f"praw{c}_{j}") for j in range(NP_)]
                    for c in range(NCHUNK)]
            traw = [[rawp.tile([P, TP * 9], f32, tag=f"traw{c}_{j}",
                               name=f"traw{c}_{j}") for j in range(NP_)]
                    for c in range(NCHUNK)]
            D = rawp.tile([P, 9, T], f16, tag="D")          # mse diff (shared)
            Z = [pl.tile([P, 9, L], f16, tag=f"Z{c}", name=f"Z{c}") for c in range(NCHUNK)]
            C = [pl.tile([P, 9, L], f16, tag=f"C{c}", name=f"C{c}") for c in range(NCHUNK)]
            W = pl.tile([P, 9, L], f16, tag="W")            # shared scratch
            M3 = pl.tile([P, 3, L], f32, tag="M3")          # shared scratch
            dd = [pl.tile([P, L], f32, tag=f"d{c}", name=f"d{c}") for c in range(NCHUNK)]
            bb = [pl.tile([P, L], f16, tag=f"b{c}", name=f"b{c}") for c in range(NCHUNK)]
            aa = [pl.tile([P, L], f16, tag=f"am{c}", name=f"am{c}") for c in range(NCHUNK)]
            sg = [pl.tile([P, L], f16, tag=f"sg{c}", name=f"sg{c}") for c in range(NCHUNK)]

            def load(ch):
                # DMA raw chunk in row-pieces; nc.sync DMAs run FIFO in
                # emission order at full BW, so chunk-0 pieces land first.
                for pc in range(NP_):
                    r0, r1 = ch*T + pc*TP, ch*T + (pc+1)*TP
                    nc.sync.dma_start(out=praw[ch][pc], in_=predv[:, r0:r1, :])
                    nc.sync.dma_start(out=traw[ch][pc], in_=targv[:, r0:r1, :])

            def deint(ch, dve_half=False):
                # copy-cast deinterleave into planes (piece-major, matching
                # DMA landing order): Z[:, comp, 0:T]=pred, [T:L]=target.
                # dve_half: route target-tensor copies to the (startup-idle)
                # Vector engine instead of ACT.
                for pc in range(NP_):
                    n0 = pc * TP
                    for raws, half in ((praw[ch], 0), (traw[ch], 1)):
                        rv = raws[pc].rearrange("p (n c) -> p n c", c=9)
                        xi = bass_mod.AP(tensor=rv.tensor, offset=rv.offset,
                                         ap=[rv.ap[0], rv.ap[2], rv.ap[1]])
                        o = Z[ch][:, :, half*T+n0:half*T+n0+TP]
                        if dve_half and half == 1:
                            nc.vector.tensor_copy(out=o, in_=xi)
                        else:
                            act(o, xi, "Copy")

            def mse(ch):
                sub(D[:, :, :], Z[ch][:, :, 0:T], Z[ch][:, :, T:L])
                act(D[:, :, :], D[:, :, :], "Square",
                    accum_out=acc[:, ch:ch+1])

            def cof_det_act(ch, it):
                first = it == 0
                last = it == K_ITERS - 1
                z, c, w = Z[ch], C[ch], W
                if not first:
                    zf = z.rearrange("p c n -> p (c n)")
                    nc.vector.tensor_scalar(out=zf, in0=zf,
                                            scalar1=CLAMP_IT, scalar2=-CLAMP_IT,
                                            op0=Alu.min, op1=Alu.max)
                # cofactors: C[i,j] = z[i1,j1]z[i2,j2] - z[i1,j2]z[i2,j1]
                # pair j in {0,1} batched (regular plane strides), j=2 single
                for i in range(3):
                    i1, i2 = (i + 1) % 3, (i + 2) % 3
                    # pairs (i,0),(i,1):
                    A1 = _pair_ap(z, 3*i1 + 1, 1, 2)
                    A2 = _pair_ap(z, 3*i2 + 2, -2, 2)
                    A3 = _pair_ap(z, 3*i1 + 2, -2, 2)
                    A4 = _pair_ap(z, 3*i2 + 1, 1, 2)
                    o1 = _pair_ap(w, 3*i, 1, 2)
                    o2 = _pair_ap(c, 3*i, 1, 2)
                    mul(o1, A1, A2)
                    mul(o2, A3, A4)
                    sub(o2, o1, o2)
                # singles (i,2): rows 0,1 pair cross-row (stride 3 / -6),
                # row 2 alone
                A1 = _pair_ap(z, 3, 3, 2)    # z[1,0], z[2,0]
                A2 = _pair_ap(z, 7, -6, 2)   # z[2,1], z[0,1]
                A3 = _pair_ap(z, 4, 3, 2)    # z[1,1], z[2,1]
                A4 = _pair_ap(z, 6, -6, 2)   # z[2,0], z[0,0]
                o1 = _pair_ap(w, 2, 3, 2)
                o2 = _pair_ap(c, 2, 3, 2)
                mul(o1, A1, A2)
                mul(o2, A3, A4)
                sub(o2, o1, o2)
                mul(w[:, 8, :], z[:, 0, :], z[:, 4, :])
                mul(c[:, 8, :], z[:, 1, :], z[:, 3, :])
                sub(c[:, 8, :], w[:, 8, :], c[:, 8, :])
                # det (fp32): d = sum_j z[0,j]*C[0,j]
                mul(M3[:, :, :], z[:, 0:3, :], c[:, 0:3, :])
                add(dd[ch], M3[:, 0, :], M3[:, 1, :])
                add(dd[ch], dd[ch], M3[:, 2, :])
                # ACT chain: a = sign(d)*|d|^(-1/3) [, b = |d|^(-2/3) on last]
                # Sign first, then Square/Ln run in place on dd.
                ofs = _c(nc, LN_HALF) if last else bias0[:, 0:1]
                act(sg[ch], dd[ch], "Sign")
                act(dd[ch], dd[ch], "Square")
                act(dd[ch], dd[ch], "Ln", bias=_c(nc, EPS_D))
                if last:
                    act(bb[ch], dd[ch], "Exp", scale=-1.0/3.0, bias=ofs)
                act(aa[ch], dd[ch], "Exp", scale=-1.0/6.0, bias=ofs)

            def update(ch, it):
                last = it == K_ITERS - 1
                z, c = Z[ch], C[ch]
                mul(sg[ch], sg[ch], aa[ch])         # sg <- a16 (signed)
                zf = z.rearrange("p c n -> p (c n)")
                cf = c.rearrange("p c n -> p (c n)")
                if not last:
                    # drift update: Z <- Z + a*C (scale absorbed by next det)
                    mul(c[:, :, :], c[:, :, :], _bc(sg[ch], 9))
                    add(zf, zf, cf)
                else:
                    mul(W[:, :, :], z[:, :, :], _bc(sg[ch], 9))
                    mul(c[:, :, :], c[:, :, :], _bc(bb[ch], 9))
                    add(zf, W.rearrange("p c n -> p (c n)"), cf)

            def rot(ch, buf, col, halves=1):
                # clamp junk rows, diff pred vs target halves, ACT sq-accum
                z = Z[ch]
                zf = z.rearrange("p c n -> p (c n)")
                nc.vector.tensor_scalar(out=zf, in0=zf, scalar1=8.0,
                                        scalar2=-8.0, op0=Alu.min, op1=Alu.max)
                for h in range(halves):
                    c0, c1 = h * 9 // halves, (h + 1) * 9 // halves
                    sub(buf[:, c0:c1, 0:T], z[:, c0:c1, 0:T],
                        z[:, c0:c1, T:L])
                    act(buf[:, c0:c1, 0:T], buf[:, c0:c1, 0:T], "Square",
                        accum_out=acc[:, col+h:col+h+1])

            # ---- software-pipelined schedule over the two chunks ----
            load(0)
            load(1)
            deint(0, dve_half=True)
            mse(0)
            deint(1)
            cof_det_act(0, 0)
            mse(1)
            cof_det_act(1, 0)
            update(0, 0)
            update(1, 0)
            for it in range(1, K_ITERS):
                cof_det_act(0, it)
                cof_det_act(1, it)
                update(0, it)
                if it == K_ITERS - 1:
                    rot(0, D, 2)        # chunk-0 rot overlaps chunk-1 update
                update(1, it)
            rot(1, W, 3, halves=2)      # split so ACT accum overlaps the sub
            nc.sync.dma_start(out=out[:, :], in_=acc)
    return nc


def _elide_implied_waits(nc):
    """Drop semaphore waits already implied by program order or transitively
    by earlier waits (vector-clock propagation).  Tile's per-instruction wait
    emission is not transitively minimal, and walrus can encode only one sync
    wait on Activation/DMA instructions (and ~4 on control instructions), so
    the redundant waits both break codegen and waste sequencer time.

    Model: each semaphore s carries a snapshot VC at every increment value;
    an engine's observed VC advances via its own instruction stream and via
    the snapshots of the waits it executes.  A wait (s >= v) is dropped iff
    the engine's observed VC already dominates it.  Unknown update modes
    disable elision for that semaphore (conservative).
    """
    join = lambda a, b: {k: max(a.get(k, 0), b.get(k, 0)) for k in set(a) | set(b)}
    sem_val = {}        # sem name -> current value
    sem_snap = {}       # sem name -> list of (value, VC) snapshots
    eng_vc = {}         # engine name -> observed VC
    unsafe = set()      # sems with non-increment updates
    n_drop = 0
    for f in nc.m.functions:
        for bb in f.blocks:
            for ins in bb.instructions:
                eng = str(ins.engine)
                vc = dict(eng_vc.get(eng, {}))
                si = ins.sync_info
                waits = list(si.on_wait) if si is not None and si.on_wait else []
                kept = []
                for w in waits:
                    s, v = w.ant_name, w.wait_value
                    if w.wait_mode != "sem-ge-imm" or s in unsafe:
                        kept.append(w)
                        continue
                    if vc.get(s, 0) >= v:
                        n_drop += 1
                        continue
                    if sem_val.get(s, 0) < v:
                        kept.append(w)
                        continue
                    kept.append(w)
                    snap = {}
                    for sv, svc in sem_snap.get(s, ()):
                        if sv <= v:
                            snap = svc
                        else:
                            break
                    vc = join(vc, snap)
                    vc[s] = max(vc.get(s, 0), v)
                if si is not None and len(kept) != len(waits):
                    si.on_wait = kept
                ups = si.on_update if si is not None and si.on_update else []
                for u in ups:
                    s = u.ant_name
                    if u.update_mode not in ("sem-inc", "sem-add-imm"):
                        unsafe.add(s)
                        continue
                    nv = sem_val.get(s, 0) + (u.update_value or 1)
                    sem_val[s] = nv
                    lst = sem_snap.setdefault(s, [])
                    prev = lst[-1][1] if lst else {}
                    lst.append((nv, join(prev, vc)))
                    if "DMA" not in s:
                        vc[s] = max(vc.get(s, 0), nv)
                eng_vc[eng] = vc
    return n_drop


_NC_CACHE = None


def kernel(pred: np.ndarray, target: np.ndarray) -> np.ndarray:
    global _NC_CACHE
    from concourse.bass_utils import run_bass_kernel_spmd

    pred = np.ascontiguousarray(np.asarray(pred, dtype=np.float32))
    target = np.ascontiguousarray(np.asarray(target, dtype=np.float32))
    assert pred.shape == (B, 9) and target.shape == (B, 9)

    if _NC_CACHE is None:
        _NC_CACHE = _build_nc()
        _elide_implied_waits(_NC_CACHE)
    nc = _NC_CACHE

    ps = pred.reshape(N_CORES, ROWS_PER_CORE, 9)
    ts = target.reshape(N_CORES, ROWS_PER_CORE, 9)
    in_maps = [{"pred": ps[i], "target": ts[i]} for i in range(N_CORES)]
    res = run_bass_kernel_spmd(nc, in_maps, core_ids=list(range(N_CORES)))
    globals()["_LAST_RESULT"] = res

    mse_sum = 0.0
    rot_sum = 0.0
    for r in res.results:
        part = np.asarray(r["partials"], dtype=np.float64)
        mse_sum += part[:, :2].sum()
        rot_sum += part[:, 2:].sum()
    n = float(B * 9)
    return np.asarray(np.float32(mse_sum / n + 0.5 * (rot_sum / n)))


# revision 22
# speedup vs baseline: 2.0918x; 1.0183x over previous
"""CustomPoseLoss Trainium2 kernel.

loss = mean((pred-target)^2) + 0.5 * mean((R(pred)-R(target))^2)
where R(M) = sign(det M) * polar(M) for each 3x3 matrix (row of 9).

Implementation: det-scaled Newton iteration for the polar factor, with the
sign fix folded into the first iteration's scaling (R = polar(sign(det M)*M),
and the signed cube root sign(d)*|d|^{-1/3} handles it for free):

  Z_0 = M
  Z_{k+1} = a_k * Z_k + b_k * cof(Z_k),   a = d^{-1/3}, b = d^{-2/3}
  (d = det Z_k; the 0.5 Newton averaging is deferred into the scaling and
   applied via a ln(0.5) bias on the final iteration's exponentials)

All plane arithmetic is f16 unit-stride so DVE tensor_tensor runs in 2x mode;
det is accumulated in fp32 (avoids f16 inf -> NaN); iterates are clamped to
+-180 before each cofactor pass so every f16 product stays below 65504.
The transcendental chain (Square/Ln/Exp/Sign) runs on the Scalar engine in
parallel with the Vector engine's cofactor work of the other chunk
(two chunks are software-pipelined for exactly this overlap).

Sharding: pure data parallel over 8 cores; each core reduces its shard to
[128, 2*NCHUNK] partial sums (mse, rot), host combines in float64.
"""

import numpy as np

B = 1048576
N_CORES = 8
ROWS_PER_CORE = B // N_CORES          # 131072
P = 128
ROWS_PER_PART = ROWS_PER_CORE // P    # 1024
T = 512                               # rows per partition per chunk (per tensor)
NCHUNK = ROWS_PER_PART // T           # 2
L = 2 * T                             # plane width: [pred rows | target rows]
K_ITERS = 3
CLAMP_IT = 180.0
EPS_D = 1e-7
LN_HALF = float(np.log(0.5))

_CONST_STATE = {}
bass_mod = None


def _c(nc, v):
    """[P,1] fp32 constant AP, DVE-memset once (keeps ACT single-wait)."""
    key = float(np.float32(v))
    consts = _CONST_STATE.setdefault(id(nc), {})
    if key not in consts:
        pool = _CONST_STATE[(id(nc), "pool")]
        from concourse import mybir
        t = pool.tile([P, 1], mybir.dt.float32, tag=f"c{len(consts)}", name=f"c{len(consts)}")
        nc.vector.memset(t, key)
        consts[key] = t
    return consts[key][:, 0:1]


def _plane_do(tile):
    return tile[:, 1, :].offset - tile[:, 0, :].offset


def _pair_ap(tile, k0, stride_planes, n):
    """AP over n planes of `tile` ([P, 9, L] f16) starting at plane k0 with a
    plane-stride of `stride_planes` (may be negative)."""
    p0 = tile[:, k0, :]
    do = _plane_do(tile)
    return bass_mod.AP(tensor=p0.tensor, offset=p0.offset,
                       ap=[p0.ap[0], [do * stride_planes, n], p0.ap[1]])


def _quad_ap(tile, k0, s_row, s_col):
    """4D AP: 2x2 grid of planes starting at k0 with plane-strides
    (s_row, s_col)."""
    p0 = tile[:, k0, :]
    do = _plane_do(tile)
    return bass_mod.AP(tensor=p0.tensor, offset=p0.offset,
                       ap=[p0.ap[0], [do * s_row, 2], [do * s_col, 2],
                           p0.ap[1]])


def _bc(plane, k):
    """broadcast [P, L] plane across k planes -> [P, k, L]"""
    return bass_mod.AP(tensor=plane.tensor, offset=plane.offset,
                       ap=[plane.ap[0], [0, k], plane.ap[1]])


def _build_nc():
    global bass_mod
    import concourse.bass as bass
    import concourse.tile as tile
    from concourse import mybir
    bass_mod = bass

    f32 = mybir.dt.float32
    f16 = mybir.dt.float16
    Alu = mybir.AluOpType
    Act = mybir.ActivationFunctionType

    nc = bass.Bass()
    pred = nc.dram_tensor("pred", [ROWS_PER_CORE, 9], f32, kind="ExternalInput")
    targ = nc.dram_tensor("target", [ROWS_PER_CORE, 9], f32, kind="ExternalInput")
    out = nc.dram_tensor("partials", [P, 2 + NCHUNK + 1], f32,
                         kind="ExternalOutput")

    predv = pred.rearrange("(p n) c -> p n c", p=P)    # [128, 1024, 9]
    targv = targ.rearrange("(p n) c -> p n c", p=P)

    def mul(o, a, b):
        nc.vector.tensor_tensor(out=o, in0=a, in1=b, op=Alu.mult)

    def add(o, a, b):
        nc.vector.tensor_tensor(out=o, in0=a, in1=b, op=Alu.add)

    def sub(o, a, b):
        nc.vector.tensor_tensor(out=o, in0=a, in1=b, op=Alu.subtract)

    with tile.TileContext(nc) as tc:
        with (
            tc.tile_pool(name="raw", bufs=1) as rawp,
            tc.tile_pool(name="pl", bufs=1) as pl,
            tc.tile_pool(name="acc", bufs=1) as accp,
        ):
            acc = accp.tile([P, 2 + NCHUNK + 1], f32, tag="acc")
            bias0 = accp.tile([P, 1], f32, tag="bias0")
            nc.vector.memset(bias0, 0.0)
            _CONST_STATE[(id(nc), "pool")] = accp

            def act(o, a, func, scale=1.0, bias=None, accum_out=None):
                if func == "Copy":
                    nc.scalar.activation(out=o, in_=a, func=Act.Copy,
                                         bias=0.0, scale=float(scale),
                                         accum_out=accum_out)
                else:
                    nc.scalar.activation(
                        out=o, in_=a, func=getattr(Act, func),
                        bias=bias0[:, 0:1] if bias is None else bias,
                        scale=float(scale), accum_out=accum_out)

            NP_ = 4   # DMA pieces per tensor-chunk (separate tiles so each
            TP = T // NP_   # deint copy waits only on its own piece's DMA)
            praw = [[rawp.tile([P, TP * 9], f32, tag=f"praw{c}_{j}",
                               name=f"praw{c}_{j}") for j in range(NP_)]
                    for c in range(NCHUNK)]
            traw = [[rawp.tile([P, TP * 9], f32, tag=f"traw{c}_{j}",
                               name=f"traw{c}_{j}") for j in range(NP_)]
                    for c in range(NCHUNK)]
            D = rawp.tile([P, 9, T], f16, tag="D")          # mse diff (shared)
            Z = [pl.tile([P, 9, L], f16, tag=f"Z{c}", name=f"Z{c}") for c in range(NCHUNK)]
            C = [pl.tile([P, 9, L], f16, tag=f"C{c}", name=f"C{c}") for c in range(NCHUNK)]
            W = pl.tile([P, 9, L], f16, tag="W")            # shared scratch
            M3 = pl.tile([P, 3, L], f32, tag="M3")          # shared scratch
            dd = [pl.tile([P, L], f32, tag=f"d{c}", name=f"d{c}") for c in range(NCHUNK)]
            bb = [pl.tile([P, L], f16, tag=f"b{c}", name=f"b{c}") for c in range(NCHUNK)]
            aa = [pl.tile([P, L], f16, tag=f"am{c}", name=f"am{c}") for c in range(NCHUNK)]
            sg = [pl.tile([P, L], f16, tag=f"sg{c}", name=f"sg{c}") for c in range(NCHUNK)]

            def load(ch):
                # DMA raw chunk in row-pieces; nc.sync DMAs run FIFO in
                # emission order at full BW, so chunk-0 pieces land first.
                for pc in range(NP_):
                    r0, r1 = ch*T + pc*TP, ch*T + (pc+1)*TP
                    nc.sync.dma_start(out=praw[ch][pc], in_=predv[:, r0:r1, :])
                    nc.sync.dma_start(out=traw[ch][pc], in_=targv[:, r0:r1, :])

            def deint(ch, dve_half=False):
                # copy-cast deinterleave into planes (piece-major, matching
                # DMA landing order): Z[:, comp, 0:T]=pred, [T:L]=target.
                # dve_half: route target-tensor copies to the (startup-idle)
                # Vector engine instead of ACT.
                for pc in range(NP_):
                    n0 = pc * TP
                    for raws, half in ((praw[ch], 0), (traw[ch], 1)):
                        rv = raws[pc].rearrange("p (n c) -> p n c", c=9)
                        xi = bass_mod.AP(tensor=rv.tensor, offset=rv.offset,
                                         ap=[rv.ap[0], rv.ap[2], rv.ap[1]])
                        o = Z[ch][:, :, half*T+n0:half*T+n0+TP]
                        if dve_half and half == 1:
                            nc.vector.tensor_copy(out=o, in_=xi)
                        else:
                            act(o, xi, "Copy")

            def mse(ch):
                sub(D[:, :, :], Z[ch][:, :, 0:T], Z[ch][:, :, T:L])
                act(D[:, :, :], D[:, :, :], "Square",
                    accum_out=acc[:, ch:ch+1])

            def cof_det_act(ch, it):
                first = it == 0
                last = it == K_ITERS - 1
                z, c, w = Z[ch], C[ch], W
                if not first:
                    zf = z.rearrange("p c n -> p (c n)")
                    nc.vector.tensor_scalar(out=zf, in0=zf,
                                            scalar1=CLAMP_IT, scalar2=-CLAMP_IT,
                                            op0=Alu.min, op1=Alu.max)
                # cofactors: C[i,j] = z[i1,j1]z[i2,j2] - z[i1,j2]z[i2,j1]
                # rows 0,1 x cols 0,1 as one 4D-batched quad (row-stride,
                # col-stride regular); row 2 cols {0,1} as a pair; j=2 column
                # cross-paired; (2,2) single
                mul(_quad_ap(w, 0, 3, 1), _quad_ap(z, 4, 3, 1),
                    _quad_ap(z, 8, -6, -2))
                mul(_quad_ap(c, 0, 3, 1), _quad_ap(z, 5, 3, -2),
                    _quad_ap(z, 7, -6, 1))
                sub(_quad_ap(c, 0, 3, 1), _quad_ap(w, 0, 3, 1),
                    _quad_ap(c, 0, 3, 1))
                for i in (2,):
                    i1, i2 = (i + 1) % 3, (i + 2) % 3
                    # pairs (i,0),(i,1):
                    A1 = _pair_ap(z, 3*i1 + 1, 1, 2)
                    A2 = _pair_ap(z, 3*i2 + 2, -2, 2)
                    A3 = _pair_ap(z, 3*i1 + 2, -2, 2)
                    A4 = _pair_ap(z, 3*i2 + 1, 1, 2)
                    o1 = _pair_ap(w, 3*i, 1, 2)
                    o2 = _pair_ap(c, 3*i, 1, 2)
                    mul(o1, A1, A2)
                    mul(o2, A3, A4)
                    sub(o2, o1, o2)
                # singles (i,2): rows 0,1 pair cross-row (stride 3 / -6),
                # row 2 alone
                A1 = _pair_ap(z, 3, 3, 2)    # z[1,0], z[2,0]
                A2 = _pair_ap(z, 7, -6, 2)   # z[2,1], z[0,1]
                A3 = _pair_ap(z, 4, 3, 2)    # z[1,1], z[2,1]
                A4 = _pair_ap(z, 6, -6, 2)   # z[2,0], z[0,0]
                o1 = _pair_ap(w, 2, 3, 2)
                o2 = _pair_ap(c, 2, 3, 2)
                mul(o1, A1, A2)
                mul(o2, A3, A4)
                sub(o2, o1, o2)
                mul(w[:, 8, :], z[:, 0, :], z[:, 4, :])
                mul(c[:, 8, :], z[:, 1, :], z[:, 3, :])
                sub(c[:, 8, :], w[:, 8, :], c[:, 8, :])
                # det: d = sum_j z[0,j]*C[0,j].
                # iter 1: fp32 (heavy cancellation in det of raw Gaussians);
                # iters 2+: f16 with exact 2^-8 prescale (terms same-sign,
                # keeps every f16 product/sum below 65504 for clamped junk
                # rows), Square's free scale=256 restores the magnitude.
                ofs = _c(nc, LN_HALF) if last else bias0[:, 0:1]
                if first:
                    mul(M3[:, :, :], z[:, 0:3, :], c[:, 0:3, :])
                    add(dd[ch], M3[:, 0, :], M3[:, 1, :])
                    add(dd[ch], dd[ch], M3[:, 2, :])
                    act(sg[ch], dd[ch], "Sign")
                    act(dd[ch], dd[ch], "Square")
                else:
                    nc.vector.tensor_scalar(out=w[:, 0:3, :], in0=z[:, 0:3, :],
                                            scalar1=2.0**-8, scalar2=None,
                                            op0=Alu.mult)
                    mul(w[:, 3:6, :], w[:, 0:3, :], c[:, 0:3, :])
                    add(w[:, 6, :], w[:, 3, :], w[:, 4, :])
                    add(w[:, 7, :], w[:, 6, :], w[:, 5, :])
                    act(sg[ch], w[:, 7, :], "Sign")
                    act(dd[ch], w[:, 7, :], "Square", scale=256.0)
                act(dd[ch], dd[ch], "Ln", bias=_c(nc, EPS_D))
                if last:
                    act(bb[ch], dd[ch], "Exp", scale=-1.0/3.0, bias=ofs)
                act(aa[ch], dd[ch], "Exp", scale=-1.0/6.0, bias=ofs)

            def update(ch, it):
                last = it == K_ITERS - 1
                z, c = Z[ch], C[ch]
                mul(sg[ch], sg[ch], aa[ch])         # sg <- a16 (signed)
                zf = z.rearrange("p c n -> p (c n)")
                cf = c.rearrange("p c n -> p (c n)")
                if not last:
                    # drift update: Z <- Z + a*C (scale absorbed by next det)
                    mul(c[:, :, :], c[:, :, :], _bc(sg[ch], 9))
                    add(zf, zf, cf)
                else:
                    mul(W[:, :, :], z[:, :, :], _bc(sg[ch], 9))
                    mul(c[:, :, :], c[:, :, :], _bc(bb[ch], 9))
                    add(zf, W.rearrange("p c n -> p (c n)"), cf)

            def rot(ch, buf, col, halves=1):
                # clamp junk rows, diff pred vs target halves, ACT sq-accum
                z = Z[ch]
                zf = z.rearrange("p c n -> p (c n)")
                nc.vector.tensor_scalar(out=zf, in0=zf, scalar1=8.0,
                                        scalar2=-8.0, op0=Alu.min, op1=Alu.max)
                for h in range(halves):
                    c0, c1 = h * 9 // halves, (h + 1) * 9 // halves
                    sub(buf[:, c0:c1, 0:T], z[:, c0:c1, 0:T],
                        z[:, c0:c1, T:L])
                    act(buf[:, c0:c1, 0:T], buf[:, c0:c1, 0:T], "Square",
                        accum_out=acc[:, col+h:col+h+1])

            # ---- software-pipelined schedule over the two chunks ----
            load(0)
            load(1)
            deint(0, dve_half=True)
            mse(0)
            deint(1)
            cof_det_act(0, 0)
            mse(1)
            cof_det_act(1, 0)
            update(0, 0)
            update(1, 0)
            for it in range(1, K_ITERS):
                cof_det_act(0, it)
                cof_det_act(1, it)
                update(0, it)
                if it == K_ITERS - 1:
                    rot(0, D, 2)        # chunk-0 rot overlaps chunk-1 update
                update(1, it)
            rot(1, W, 3, halves=2)      # split so ACT accum overlaps the sub
            nc.sync.dma_start(out=out[:, :], in_=acc)
    return nc


def _elide_implied_waits(nc):
    """Drop semaphore waits already implied by program order or transitively
    by earlier waits (vector-clock propagation).  Tile's per-instruction wait
    emission is not transitively minimal, and walrus can encode only one sync
    wait on Activation/DMA instructions (and ~4 on control instructions), so
    the redundant waits both break codegen and waste sequencer time.

    Model: each semaphore s carries a snapshot VC at every increment value;
    an engine's observed VC advances via its own instruction stream and via
    the snapshots of the waits it executes.  A wait (s >= v) is dropped iff
    the engine's observed VC already dominates it.  Unknown update modes
    disable elision for that semaphore (conservative).
    """
    join = lambda a, b: {k: max(a.get(k, 0), b.get(k, 0)) for k in set(a) | set(b)}
    sem_val = {}        # sem name -> current value
    sem_snap = {}       # sem name -> list of (value, VC) snapshots
    eng_vc = {}         # engine name -> observed VC
    unsafe = set()      # sems with non-increment updates
    n_drop = 0
    for f in nc.m.functions:
        for bb in f.blocks:
            for ins in bb.instructions:
                eng = str(ins.engine)
                vc = dict(eng_vc.get(eng, {}))
                si = ins.sync_info
                waits = list(si.on_wait) if si is not None and si.on_wait else []
                kept = []
                for w in waits:
                    s, v = w.ant_name, w.wait_value
                    if w.wait_mode != "sem-ge-imm" or s in unsafe:
                        kept.append(w)
                        continue
                    if vc.get(s, 0) >= v:
                        n_drop += 1
                        continue
                    if sem_val.get(s, 0) < v:
                        kept.append(w)
                        continue
                    kept.append(w)
                    snap = {}
                    for sv, svc in sem_snap.get(s, ()):
                        if sv <= v:
                            snap = svc
                        else:
                            break
                    vc = join(vc, snap)
                    vc[s] = max(vc.get(s, 0), v)
                if si is not None and len(kept) != len(waits):
                    si.on_wait = kept
                ups = si.on_update if si is not None and si.on_update else []
                for u in ups:
                    s = u.ant_name
                    if u.update_mode not in ("sem-inc", "sem-add-imm"):
                        unsafe.add(s)
                        continue
                    nv = sem_val.get(s, 0) + (u.update_value or 1)
                    sem_val[s] = nv
                    lst = sem_snap.setdefault(s, [])
                    prev = lst[-1][1] if lst else {}
                    lst.append((nv, join(prev, vc)))
                    if "DMA" not in s:
                        vc[s] = max(vc.get(s, 0), nv)
                eng_vc[eng] = vc
    return n_drop


_NC_CACHE = None


def kernel(pred: np.ndarray, target: np.ndarray) -> np.ndarray:
    global _NC_CACHE
    from concourse.bass_utils import run_bass_kernel_spmd

    pred = np.ascontiguousarray(np.asarray(pred, dtype=np.float32))
    target = np.ascontiguousarray(np.asarray(target, dtype=np.float32))
    assert pred.shape == (B, 9) and target.shape == (B, 9)

    if _NC_CACHE is None:
        _NC_CACHE = _build_nc()
        _elide_implied_waits(_NC_CACHE)
    nc = _NC_CACHE

    ps = pred.reshape(N_CORES, ROWS_PER_CORE, 9)
    ts = target.reshape(N_CORES, ROWS_PER_CORE, 9)
    in_maps = [{"pred": ps[i], "target": ts[i]} for i in range(N_CORES)]
    res = run_bass_kernel_spmd(nc, in_maps, core_ids=list(range(N_CORES)))
    globals()["_LAST_RESULT"] = res

    mse_sum = 0.0
    rot_sum = 0.0
    for r in res.results:
        part = np.asarray(r["partials"], dtype=np.float64)
        mse_sum += part[:, :2].sum()
        rot_sum += part[:, 2:].sum()
    n = float(B * 9)
    return np.asarray(np.float32(mse_sum / n + 0.5 * (rot_sum / n)))


# revision 23
# speedup vs baseline: 2.1051x; 1.0064x over previous
"""CustomPoseLoss Trainium2 kernel.

loss = mean((pred-target)^2) + 0.5 * mean((R(pred)-R(target))^2)
where R(M) = sign(det M) * polar(M) for each 3x3 matrix (row of 9).

Implementation: det-scaled Newton iteration for the polar factor (K=3).
The sign fix folds into the scaling: R = polar(sign(det M)*M), handled by
using the signed cube root a = sign(d)*|d|^{-1/3} each iteration.

  non-final iterations (drift form, 2 big ops instead of 3):
      Z <- Z + a*cof(Z)
    The per-sample scale drifts by 1/a, but determinant scaling absorbs any
    per-sample scalar at the next iteration, so only the final iteration
    normalizes:
      Z_K = a*Z + a^2*cof(Z),  with a 0.5 Newton-averaging factor applied
    free of charge via a ln(0.5) bias on the final Exp activations.

All plane arithmetic is f16 unit-stride so DVE tensor_tensor runs in 2x mode
(cofactors batched as one 2x2-plane 4D-AP quad + pairs via regular +-plane
strides); iterates are clamped to +-180 before each cofactor pass so every
f16 product stays below 65504 (no inf-inf => no NaN by construction).
det: fp32 at iteration 1 (raw Gaussian dets cancel heavily), f16 with an
exact 2^-8 prescale afterwards (terms are same-signed; Square's free scale
restores magnitude).  The transcendental chain (Sign/Square/Ln/Exp) runs on
the Scalar engine overlapped with the Vector engine's cofactor work of the
other chunk (two chunks software-pipelined); the deinterleave copy-casts run
on ACT (pred half) and the startup-idle DVE (target half), pipelined behind
piecewise DMA.

Sharding: pure data parallel over 8 cores; each core reduces its shard to
[128, 5] partial sums (2x mse, 3x rot), host combines in float64.
"""

import numpy as np

B = 1048576
N_CORES = 8
ROWS_PER_CORE = B // N_CORES          # 131072
P = 128
ROWS_PER_PART = ROWS_PER_CORE // P    # 1024
T = 512                               # rows per partition per chunk (per tensor)
NCHUNK = ROWS_PER_PART // T           # 2
L = 2 * T                             # plane width: [pred rows | target rows]
K_ITERS = 3
CLAMP_IT = 180.0
EPS_D = 1e-7
LN_HALF = float(np.log(0.5))

_CONST_STATE = {}
bass_mod = None


def _c(nc, v):
    """[P,1] fp32 constant AP, DVE-memset once (keeps ACT single-wait)."""
    key = float(np.float32(v))
    consts = _CONST_STATE.setdefault(id(nc), {})
    if key not in consts:
        pool = _CONST_STATE[(id(nc), "pool")]
        from concourse import mybir
        t = pool.tile([P, 1], mybir.dt.float32, tag=f"c{len(consts)}", name=f"c{len(consts)}")
        nc.vector.memset(t, key)
        consts[key] = t
    return consts[key][:, 0:1]


def _plane_do(tile):
    return tile[:, 1, :].offset - tile[:, 0, :].offset


def _pair_ap(tile, k0, stride_planes, n):
    """AP over n planes of `tile` ([P, 9, L] f16) starting at plane k0 with a
    plane-stride of `stride_planes` (may be negative)."""
    p0 = tile[:, k0, :]
    do = _plane_do(tile)
    return bass_mod.AP(tensor=p0.tensor, offset=p0.offset,
                       ap=[p0.ap[0], [do * stride_planes, n], p0.ap[1]])


def _quad_ap(tile, k0, s_row, s_col):
    """4D AP: 2x2 grid of planes starting at k0 with plane-strides
    (s_row, s_col)."""
    p0 = tile[:, k0, :]
    do = _plane_do(tile)
    return bass_mod.AP(tensor=p0.tensor, offset=p0.offset,
                       ap=[p0.ap[0], [do * s_row, 2], [do * s_col, 2],
                           p0.ap[1]])


def _bc(plane, k):
    """broadcast [P, L] plane across k planes -> [P, k, L]"""
    return bass_mod.AP(tensor=plane.tensor, offset=plane.offset,
                       ap=[plane.ap[0], [0, k], plane.ap[1]])


def _build_nc():
    global bass_mod
    import concourse.bass as bass
    import concourse.tile as tile
    from concourse import mybir
    bass_mod = bass

    f32 = mybir.dt.float32
    f16 = mybir.dt.float16
    Alu = mybir.AluOpType
    Act = mybir.ActivationFunctionType

    nc = bass.Bass()
    pred = nc.dram_tensor("pred", [ROWS_PER_CORE, 9], f32, kind="ExternalInput")
    targ = nc.dram_tensor("target", [ROWS_PER_CORE, 9], f32, kind="ExternalInput")
    out = nc.dram_tensor("partials", [P, 2 + NCHUNK + 1], f32,
                         kind="ExternalOutput")

    predv = pred.rearrange("(p n) c -> p n c", p=P)    # [128, 1024, 9]
    targv = targ.rearrange("(p n) c -> p n c", p=P)

    def mul(o, a, b):
        nc.vector.tensor_tensor(out=o, in0=a, in1=b, op=Alu.mult)

    def add(o, a, b):
        nc.vector.tensor_tensor(out=o, in0=a, in1=b, op=Alu.add)

    def sub(o, a, b):
        nc.vector.tensor_tensor(out=o, in0=a, in1=b, op=Alu.subtract)

    with tile.TileContext(nc) as tc:
        with (
            tc.tile_pool(name="raw", bufs=1) as rawp,
            tc.tile_pool(name="pl", bufs=1) as pl,
            tc.tile_pool(name="acc", bufs=1) as accp,
        ):
            acc = accp.tile([P, 2 + NCHUNK + 1], f32, tag="acc")
            bias0 = accp.tile([P, 1], f32, tag="bias0")
            nc.vector.memset(bias0, 0.0)
            _CONST_STATE[(id(nc), "pool")] = accp

            def act(o, a, func, scale=1.0, bias=None, accum_out=None):
                if func == "Copy":
                    nc.scalar.activation(out=o, in_=a, func=Act.Copy,
                                         bias=0.0, scale=float(scale),
                                         accum_out=accum_out)
                else:
                    nc.scalar.activation(
                        out=o, in_=a, func=getattr(Act, func),
                        bias=bias0[:, 0:1] if bias is None else bias,
                        scale=float(scale), accum_out=accum_out)

            NP_ = 4   # DMA pieces per tensor-chunk (separate tiles so each
            TP = T // NP_   # deint copy waits only on its own piece's DMA)
            praw = [[rawp.tile([P, TP * 9], f32, tag=f"praw{c}_{j}",
                               name=f"praw{c}_{j}") for j in range(NP_)]
                    for c in range(NCHUNK)]
            traw = [[rawp.tile([P, TP * 9], f32, tag=f"traw{c}_{j}",
                               name=f"traw{c}_{j}") for j in range(NP_)]
                    for c in range(NCHUNK)]
            D = rawp.tile([P, 9, T], f16, tag="D")          # mse diff (shared)
            Z = [pl.tile([P, 9, L], f16, tag=f"Z{c}", name=f"Z{c}") for c in range(NCHUNK)]
            C = [pl.tile([P, 9, L], f16, tag=f"C{c}", name=f"C{c}") for c in range(NCHUNK)]
            W = pl.tile([P, 9, L], f16, tag="W")            # shared scratch
            M3 = pl.tile([P, 3, L], f32, tag="M3")          # shared scratch
            dd = [pl.tile([P, L], f32, tag=f"d{c}", name=f"d{c}") for c in range(NCHUNK)]
            bb = [pl.tile([P, L], f16, tag=f"b{c}", name=f"b{c}") for c in range(NCHUNK)]
            aa = [pl.tile([P, L], f16, tag=f"am{c}", name=f"am{c}") for c in range(NCHUNK)]
            sg = [pl.tile([P, L], f16, tag=f"sg{c}", name=f"sg{c}") for c in range(NCHUNK)]

            def load(ch):
                # DMA raw chunk in row-pieces; nc.sync DMAs run FIFO in
                # emission order at full BW, so chunk-0 pieces land first.
                for pc in range(NP_):
                    r0, r1 = ch*T + pc*TP, ch*T + (pc+1)*TP
                    nc.sync.dma_start(out=praw[ch][pc], in_=predv[:, r0:r1, :])
                    nc.sync.dma_start(out=traw[ch][pc], in_=targv[:, r0:r1, :])

            def deint(ch, dve_half=False):
                # copy-cast deinterleave into planes (piece-major, matching
                # DMA landing order): Z[:, comp, 0:T]=pred, [T:L]=target.
                # dve_half: route target-tensor copies to the (startup-idle)
                # Vector engine instead of ACT.
                for pc in range(NP_):
                    n0 = pc * TP
                    for raws, half in ((praw[ch], 0), (traw[ch], 1)):
                        rv = raws[pc].rearrange("p (n c) -> p n c", c=9)
                        xi = bass_mod.AP(tensor=rv.tensor, offset=rv.offset,
                                         ap=[rv.ap[0], rv.ap[2], rv.ap[1]])
                        o = Z[ch][:, :, half*T+n0:half*T+n0+TP]
                        if dve_half and half == 1:
                            nc.vector.tensor_copy(out=o, in_=xi)
                        else:
                            act(o, xi, "Copy")

            def mse(ch):
                sub(D[:, :, :], Z[ch][:, :, 0:T], Z[ch][:, :, T:L])
                act(D[:, :, :], D[:, :, :], "Square",
                    accum_out=acc[:, ch:ch+1])

            def cof_det_act(ch, it):
                first = it == 0
                last = it == K_ITERS - 1
                z, c, w = Z[ch], C[ch], W
                if not first:
                    zf = z.rearrange("p c n -> p (c n)")
                    nc.vector.tensor_scalar(out=zf, in0=zf,
                                            scalar1=CLAMP_IT, scalar2=-CLAMP_IT,
                                            op0=Alu.min, op1=Alu.max)
                # cofactors: C[i,j] = z[i1,j1]z[i2,j2] - z[i1,j2]z[i2,j1]
                # rows 0,1 x cols 0,1 as one 4D-batched quad (row-stride,
                # col-stride regular); row 2 cols {0,1} as a pair; j=2 column
                # cross-paired; (2,2) single
                mul(_quad_ap(w, 0, 3, 1), _quad_ap(z, 4, 3, 1),
                    _quad_ap(z, 8, -6, -2))
                mul(_quad_ap(c, 0, 3, 1), _quad_ap(z, 5, 3, -2),
                    _quad_ap(z, 7, -6, 1))
                sub(_quad_ap(c, 0, 3, 1), _quad_ap(w, 0, 3, 1),
                    _quad_ap(c, 0, 3, 1))
                for i in (2,):
                    i1, i2 = (i + 1) % 3, (i + 2) % 3
                    # pairs (i,0),(i,1):
                    A1 = _pair_ap(z, 3*i1 + 1, 1, 2)
                    A2 = _pair_ap(z, 3*i2 + 2, -2, 2)
                    A3 = _pair_ap(z, 3*i1 + 2, -2, 2)
                    A4 = _pair_ap(z, 3*i2 + 1, 1, 2)
                    o1 = _pair_ap(w, 3*i, 1, 2)
                    o2 = _pair_ap(c, 3*i, 1, 2)
                    mul(o1, A1, A2)
                    mul(o2, A3, A4)
                    sub(o2, o1, o2)
                # singles (i,2): rows 0,1 pair cross-row (stride 3 / -6),
                # row 2 alone
                A1 = _pair_ap(z, 3, 3, 2)    # z[1,0], z[2,0]
                A2 = _pair_ap(z, 7, -6, 2)   # z[2,1], z[0,1]
                A3 = _pair_ap(z, 4, 3, 2)    # z[1,1], z[2,1]
                A4 = _pair_ap(z, 6, -6, 2)   # z[2,0], z[0,0]
                o1 = _pair_ap(w, 2, 3, 2)
                o2 = _pair_ap(c, 2, 3, 2)
                mul(o1, A1, A2)
                mul(o2, A3, A4)
                sub(o2, o1, o2)
                mul(w[:, 8, :], z[:, 0, :], z[:, 4, :])
                mul(c[:, 8, :], z[:, 1, :], z[:, 3, :])
                sub(c[:, 8, :], w[:, 8, :], c[:, 8, :])
                # det: d = sum_j z[0,j]*C[0,j].
                # iter 1: fp32 (heavy cancellation in det of raw Gaussians);
                # iters 2+: f16 with exact 2^-8 prescale (terms same-sign,
                # keeps every f16 product/sum below 65504 for clamped junk
                # rows), Square's free scale=256 restores the magnitude.
                ofs = _c(nc, LN_HALF) if last else bias0[:, 0:1]
                if first:
                    mul(M3[:, :, :], z[:, 0:3, :], c[:, 0:3, :])
                    add(dd[ch], M3[:, 0, :], M3[:, 1, :])
                    add(dd[ch], dd[ch], M3[:, 2, :])
                    act(sg[ch], dd[ch], "Sign")
                    act(dd[ch], dd[ch], "Square")
                else:
                    nc.vector.tensor_scalar(out=w[:, 0:3, :], in0=z[:, 0:3, :],
                                            scalar1=2.0**-8, scalar2=None,
                                            op0=Alu.mult)
                    mul(w[:, 3:6, :], w[:, 0:3, :], c[:, 0:3, :])
                    add(w[:, 6, :], w[:, 3, :], w[:, 4, :])
                    add(w[:, 7, :], w[:, 6, :], w[:, 5, :])
                    act(sg[ch], w[:, 7, :], "Sign")
                    act(dd[ch], w[:, 7, :], "Square", scale=256.0)
                act(dd[ch], dd[ch], "Ln", bias=_c(nc, EPS_D))
                if last:
                    act(bb[ch], dd[ch], "Exp", scale=-1.0/3.0, bias=ofs)
                act(aa[ch], dd[ch], "Exp", scale=-1.0/6.0, bias=ofs)

            def update(ch, it):
                last = it == K_ITERS - 1
                z, c = Z[ch], C[ch]
                mul(sg[ch], sg[ch], aa[ch])         # sg <- a16 (signed)
                zf = z.rearrange("p c n -> p (c n)")
                cf = c.rearrange("p c n -> p (c n)")
                if not last:
                    # drift update: Z <- Z + a*C (scale absorbed by next det)
                    mul(c[:, :, :], c[:, :, :], _bc(sg[ch], 9))
                    add(zf, zf, cf)
                else:
                    mul(W[:, :, :], z[:, :, :], _bc(sg[ch], 9))
                    mul(c[:, :, :], c[:, :, :], _bc(bb[ch], 9))
                    add(zf, W.rearrange("p c n -> p (c n)"), cf)

            def rot(ch, buf, col, halves=1):
                # clamp junk rows, diff pred vs target halves, ACT sq-accum
                z = Z[ch]
                zf = z.rearrange("p c n -> p (c n)")
                nc.vector.tensor_scalar(out=zf, in0=zf, scalar1=8.0,
                                        scalar2=-8.0, op0=Alu.min, op1=Alu.max)
                for h in range(halves):
                    c0, c1 = h * 9 // halves, (h + 1) * 9 // halves
                    sub(buf[:, c0:c1, 0:T], z[:, c0:c1, 0:T],
                        z[:, c0:c1, T:L])
                    act(buf[:, c0:c1, 0:T], buf[:, c0:c1, 0:T], "Square",
                        accum_out=acc[:, col+h:col+h+1])

            # ---- software-pipelined schedule over the two chunks ----
            load(0)
            load(1)
            deint(0, dve_half=True)
            mse(0)
            deint(1)
            cof_det_act(0, 0)
            mse(1)
            cof_det_act(1, 0)
            update(0, 0)
            update(1, 0)
            for it in range(1, K_ITERS):
                cof_det_act(0, it)
                cof_det_act(1, it)
                update(0, it)
                if it == K_ITERS - 1:
                    rot(0, D, 2)        # chunk-0 rot overlaps chunk-1 update
                update(1, it)
            rot(1, W, 3, halves=2)      # split so ACT accum overlaps the sub
            nc.sync.dma_start(out=out[:, :], in_=acc)
    return nc


def _elide_implied_waits(nc):
    """Drop semaphore waits already implied by program order or transitively
    by earlier waits (vector-clock propagation).  Tile's per-instruction wait
    emission is not transitively minimal, and walrus can encode only one sync
    wait on Activation/DMA instructions (and ~4 on control instructions), so
    the redundant waits both break codegen and waste sequencer time.

    Model: each semaphore s carries a snapshot VC at every increment value;
    an engine's observed VC advances via its own instruction stream and via
    the snapshots of the waits it executes.  A wait (s >= v) is dropped iff
    the engine's observed VC already dominates it.  Unknown update modes
    disable elision for that semaphore (conservative).
    """
    join = lambda a, b: {k: max(a.get(k, 0), b.get(k, 0)) for k in set(a) | set(b)}
    sem_val = {}        # sem name -> current value
    sem_snap = {}       # sem name -> list of (value, VC) snapshots
    eng_vc = {}         # engine name -> observed VC
    unsafe = set()      # sems with non-increment updates
    n_drop = 0
    for f in nc.m.functions:
        for bb in f.blocks:
            for ins in bb.instructions:
                eng = str(ins.engine)
                vc = dict(eng_vc.get(eng, {}))
                si = ins.sync_info
                waits = list(si.on_wait) if si is not None and si.on_wait else []
                kept = []
                for w in waits:
                    s, v = w.ant_name, w.wait_value
                    if w.wait_mode != "sem-ge-imm" or s in unsafe:
                        kept.append(w)
                        continue
                    if vc.get(s, 0) >= v:
                        n_drop += 1
                        continue
                    if sem_val.get(s, 0) < v:
                        kept.append(w)
                        continue
                    kept.append(w)
                    snap = {}
                    for sv, svc in sem_snap.get(s, ()):
                        if sv <= v:
                            snap = svc
                        else:
                            break
                    vc = join(vc, snap)
                    vc[s] = max(vc.get(s, 0), v)
                if si is not None and len(kept) != len(waits):
                    si.on_wait = kept
                ups = si.on_update if si is not None and si.on_update else []
                for u in ups:
                    s = u.ant_name
                    if u.update_mode not in ("sem-inc", "sem-add-imm"):
                        unsafe.add(s)
                        continue
                    nv = sem_val.get(s, 0) + (u.update_value or 1)
                    sem_val[s] = nv
                    lst = sem_snap.setdefault(s, [])
                    prev = lst[-1][1] if lst else {}
                    lst.append((nv, join(prev, vc)))
                    if "DMA" not in s:
                        vc[s] = max(vc.get(s, 0), nv)
                eng_vc[eng] = vc
    return n_drop


_NC_CACHE = None


def kernel(pred: np.ndarray, target: np.ndarray) -> np.ndarray:
    global _NC_CACHE
    from concourse.bass_utils import run_bass_kernel_spmd

    pred = np.ascontiguousarray(np.asarray(pred, dtype=np.float32))
    target = np.ascontiguousarray(np.asarray(target, dtype=np.float32))
    assert pred.shape == (B, 9) and target.shape == (B, 9)

    if _NC_CACHE is None:
        _NC_CACHE = _build_nc()
        _elide_implied_waits(_NC_CACHE)
    nc = _NC_CACHE

    ps = pred.reshape(N_CORES, ROWS_PER_CORE, 9)
    ts = target.reshape(N_CORES, ROWS_PER_CORE, 9)
    in_maps = [{"pred": ps[i], "target": ts[i]} for i in range(N_CORES)]
    res = run_bass_kernel_spmd(nc, in_maps, core_ids=list(range(N_CORES)))
    globals()["_LAST_RESULT"] = res

    mse_sum = 0.0
    rot_sum = 0.0
    for r in res.results:
        part = np.asarray(r["partials"], dtype=np.float64)
        mse_sum += part[:, :2].sum()
        rot_sum += part[:, 2:].sum()
    n = float(B * 9)
    return np.asarray(np.float32(mse_sum / n + 0.5 * (rot_sum / n)))


# revision 26
# speedup vs baseline: 2.1079x; 1.0013x over previous
"""CustomPoseLoss Trainium2 kernel.

loss = mean((pred-target)^2) + 0.5 * mean((R(pred)-R(target))^2)
where R(M) = sign(det M) * polar(M) for each 3x3 matrix (row of 9).

Implementation: det-scaled Newton iteration for the polar factor (K=3).
The sign fix folds into the scaling: R = polar(sign(det M)*M), handled by
using the signed cube root a = sign(d)*|d|^{-1/3} each iteration.

  non-final iterations (drift form, 2 big ops instead of 3):
      Z <- Z + a*cof(Z)
    The per-sample scale drifts by 1/a, but determinant scaling absorbs any
    per-sample scalar at the next iteration, so only the final iteration
    normalizes:
      Z_K = a*Z + a^2*cof(Z),  with a 0.5 Newton-averaging factor applied
    free of charge via a ln(0.5) bias on the final Exp activations.

All plane arithmetic is f16 unit-stride so DVE tensor_tensor runs in 2x mode
(cofactors batched as one 2x2-plane 4D-AP quad + pairs via regular +-plane
strides); iterates are clamped to +-180 before each cofactor pass so every
f16 product stays below 65504 (no inf-inf => no NaN by construction).
det: fp32 at iteration 1 (raw Gaussian dets cancel heavily), f16 with an
exact 2^-8 prescale afterwards (terms are same-signed; Square's free scale
restores magnitude).  The transcendental chain (Sign/Square/Ln/Exp) runs on
the Scalar engine overlapped with the Vector engine's cofactor work of the
other chunk (two chunks software-pipelined); the deinterleave copy-casts run
on ACT (pred half) and the startup-idle DVE (target half), pipelined behind
piecewise DMA.

Sharding: pure data parallel over 8 cores; each core reduces its shard to
[128, 5] partial sums (2x mse, 3x rot), host combines in float64.
"""

import numpy as np

B = 1048576
N_CORES = 8
ROWS_PER_CORE = B // N_CORES          # 131072
P = 128
ROWS_PER_PART = ROWS_PER_CORE // P    # 1024
T = 512                               # rows per partition per chunk (per tensor)
NCHUNK = ROWS_PER_PART // T           # 2
L = 2 * T                             # plane width: [pred rows | target rows]
K_ITERS = 3
CLAMP_IT = 180.0
EPS_D = 1e-7
LN_HALF = float(np.log(0.5))

_CONST_STATE = {}
bass_mod = None


def _c(nc, v):
    """[P,1] fp32 constant AP, DVE-memset once (keeps ACT single-wait)."""
    key = float(np.float32(v))
    consts = _CONST_STATE.setdefault(id(nc), {})
    if key not in consts:
        pool = _CONST_STATE[(id(nc), "pool")]
        from concourse import mybir
        t = pool.tile([P, 1], mybir.dt.float32, tag=f"c{len(consts)}", name=f"c{len(consts)}")
        nc.vector.memset(t, key)
        consts[key] = t
    return consts[key][:, 0:1]


def _plane_do(tile):
    return tile[:, 1, :].offset - tile[:, 0, :].offset


def _pair_ap(tile, k0, stride_planes, n):
    """AP over n planes of `tile` ([P, 9, L] f16) starting at plane k0 with a
    plane-stride of `stride_planes` (may be negative)."""
    p0 = tile[:, k0, :]
    do = _plane_do(tile)
    return bass_mod.AP(tensor=p0.tensor, offset=p0.offset,
                       ap=[p0.ap[0], [do * stride_planes, n], p0.ap[1]])


def _quad_ap(tile, k0, s_row, s_col):
    """4D AP: 2x2 grid of planes starting at k0 with plane-strides
    (s_row, s_col)."""
    p0 = tile[:, k0, :]
    do = _plane_do(tile)
    return bass_mod.AP(tensor=p0.tensor, offset=p0.offset,
                       ap=[p0.ap[0], [do * s_row, 2], [do * s_col, 2],
                           p0.ap[1]])


def _bc(plane, k):
    """broadcast [P, L] plane across k planes -> [P, k, L]"""
    return bass_mod.AP(tensor=plane.tensor, offset=plane.offset,
                       ap=[plane.ap[0], [0, k], plane.ap[1]])


def _build_nc():
    global bass_mod
    import concourse.bass as bass
    import concourse.tile as tile
    from concourse import mybir
    bass_mod = bass

    f32 = mybir.dt.float32
    f16 = mybir.dt.float16
    Alu = mybir.AluOpType
    Act = mybir.ActivationFunctionType

    nc = bass.Bass()
    pred = nc.dram_tensor("pred", [ROWS_PER_CORE, 9], f32, kind="ExternalInput")
    targ = nc.dram_tensor("target", [ROWS_PER_CORE, 9], f32, kind="ExternalInput")
    out = nc.dram_tensor("partials", [P, 2 + NCHUNK + 1], f32,
                         kind="ExternalOutput")

    predv = pred.rearrange("(p n) c -> p n c", p=P)    # [128, 1024, 9]
    targv = targ.rearrange("(p n) c -> p n c", p=P)

    def mul(o, a, b):
        nc.vector.tensor_tensor(out=o, in0=a, in1=b, op=Alu.mult)

    def add(o, a, b):
        nc.vector.tensor_tensor(out=o, in0=a, in1=b, op=Alu.add)

    def sub(o, a, b):
        nc.vector.tensor_tensor(out=o, in0=a, in1=b, op=Alu.subtract)

    with tile.TileContext(nc) as tc:
        with (
            tc.tile_pool(name="raw", bufs=1) as rawp,
            tc.tile_pool(name="pl", bufs=1) as pl,
            tc.tile_pool(name="acc", bufs=1) as accp,
        ):
            acc = accp.tile([P, 2 + NCHUNK + 1], f32, tag="acc")
            bias0 = accp.tile([P, 1], f32, tag="bias0")
            nc.vector.memset(bias0, 0.0)
            _CONST_STATE[(id(nc), "pool")] = accp

            def act(o, a, func, scale=1.0, bias=None, accum_out=None):
                if func == "Copy":
                    nc.scalar.activation(out=o, in_=a, func=Act.Copy,
                                         bias=0.0, scale=float(scale),
                                         accum_out=accum_out)
                else:
                    nc.scalar.activation(
                        out=o, in_=a, func=getattr(Act, func),
                        bias=bias0[:, 0:1] if bias is None else bias,
                        scale=float(scale), accum_out=accum_out)

            NP_ = 4   # DMA pieces per tensor-chunk (separate tiles so each
            TP = T // NP_   # deint copy waits only on its own piece's DMA)
            praw = [[rawp.tile([P, TP * 9], f32, tag=f"praw{c}_{j}",
                               name=f"praw{c}_{j}") for j in range(NP_)]
                    for c in range(NCHUNK)]
            traw = [[rawp.tile([P, TP * 9], f32, tag=f"traw{c}_{j}",
                               name=f"traw{c}_{j}") for j in range(NP_)]
                    for c in range(NCHUNK)]
            D = rawp.tile([P, 9, T], f16, tag="D")          # mse diff (shared)
            Z = [pl.tile([P, 9, L], f16, tag=f"Z{c}", name=f"Z{c}") for c in range(NCHUNK)]
            C = [pl.tile([P, 9, L], f16, tag=f"C{c}", name=f"C{c}") for c in range(NCHUNK)]
            W = pl.tile([P, 9, L], f16, tag="W")            # shared scratch
            dd = [pl.tile([P, L], f32, tag=f"d{c}", name=f"d{c}") for c in range(NCHUNK)]
            bb = [pl.tile([P, L], f16, tag=f"b{c}", name=f"b{c}") for c in range(NCHUNK)]
            aa = [pl.tile([P, L], f16, tag=f"am{c}", name=f"am{c}") for c in range(NCHUNK)]
            sg = [pl.tile([P, L], f16, tag=f"sg{c}", name=f"sg{c}") for c in range(NCHUNK)]

            def load(ch):
                # DMA raw chunk in row-pieces; nc.sync DMAs run FIFO in
                # emission order at full BW, so chunk-0 pieces land first.
                for pc in range(NP_):
                    r0, r1 = ch*T + pc*TP, ch*T + (pc+1)*TP
                    nc.sync.dma_start(out=praw[ch][pc], in_=predv[:, r0:r1, :])
                    nc.sync.dma_start(out=traw[ch][pc], in_=targv[:, r0:r1, :])

            def deint(ch, dve_half=False):
                # copy-cast deinterleave into planes (piece-major, matching
                # DMA landing order): Z[:, comp, 0:T]=pred, [T:L]=target.
                # dve_half: route target-tensor copies to the (startup-idle)
                # Vector engine instead of ACT.
                for pc in range(NP_):
                    n0 = pc * TP
                    for raws, half in ((praw[ch], 0), (traw[ch], 1)):
                        rv = raws[pc].rearrange("p (n c) -> p n c", c=9)
                        xi = bass_mod.AP(tensor=rv.tensor, offset=rv.offset,
                                         ap=[rv.ap[0], rv.ap[2], rv.ap[1]])
                        o = Z[ch][:, :, half*T+n0:half*T+n0+TP]
                        if dve_half and half == 1:
                            nc.vector.tensor_copy(out=o, in_=xi)
                        else:
                            act(o, xi, "Copy")

            def mse(ch):
                sub(D[:, :, :], Z[ch][:, :, 0:T], Z[ch][:, :, T:L])
                act(D[:, :, :], D[:, :, :], "Square",
                    accum_out=acc[:, ch:ch+1])

            def cof_det_act(ch, it):
                first = it == 0
                last = it == K_ITERS - 1
                z, c, w = Z[ch], C[ch], W
                if not first:
                    zf = z.rearrange("p c n -> p (c n)")
                    nc.vector.tensor_scalar(out=zf, in0=zf,
                                            scalar1=CLAMP_IT, scalar2=-CLAMP_IT,
                                            op0=Alu.min, op1=Alu.max)
                # cofactors: C[i,j] = z[i1,j1]z[i2,j2] - z[i1,j2]z[i2,j1]
                # rows 0,1 x cols 0,1 as one 4D-batched quad (row-stride,
                # col-stride regular); row 2 cols {0,1} as a pair; j=2 column
                # cross-paired; (2,2) single
                mul(_quad_ap(w, 0, 3, 1), _quad_ap(z, 4, 3, 1),
                    _quad_ap(z, 8, -6, -2))
                mul(_quad_ap(c, 0, 3, 1), _quad_ap(z, 5, 3, -2),
                    _quad_ap(z, 7, -6, 1))
                for i in (2,):
                    i1, i2 = (i + 1) % 3, (i + 2) % 3
                    # pairs (i,0),(i,1):
                    A1 = _pair_ap(z, 3*i1 + 1, 1, 2)
                    A2 = _pair_ap(z, 3*i2 + 2, -2, 2)
                    A3 = _pair_ap(z, 3*i1 + 2, -2, 2)
                    A4 = _pair_ap(z, 3*i2 + 1, 1, 2)
                    mul(_pair_ap(w, 3*i, 1, 2), A1, A2)
                    mul(_pair_ap(c, 3*i, 1, 2), A3, A4)
                # singles (i,2): rows 0,1 pair cross-row (stride 3 / -6),
                # row 2 alone
                mul(_pair_ap(w, 2, 3, 2), _pair_ap(z, 3, 3, 2),
                    _pair_ap(z, 7, -6, 2))
                mul(_pair_ap(c, 2, 3, 2), _pair_ap(z, 4, 3, 2),
                    _pair_ap(z, 6, -6, 2))
                mul(w[:, 8, :], z[:, 0, :], z[:, 4, :])
                mul(c[:, 8, :], z[:, 1, :], z[:, 3, :])
                # all m1/m2 slots line up plane-for-plane -> ONE flat sub
                sub(c.rearrange("p c n -> p (c n)"),
                    w.rearrange("p c n -> p (c n)"),
                    c.rearrange("p c n -> p (c n)"))
                # det: d = sum_j z[0,j]*C[0,j].
                # iter 1: fp32 (heavy cancellation in det of raw Gaussians);
                # iters 2+: f16 with exact 2^-8 prescale (terms same-sign,
                # keeps every f16 product/sum below 65504 for clamped junk
                # rows), Square's free scale=256 restores the magnitude.
                ofs = _c(nc, LN_HALF) if last else bias0[:, 0:1]
                if first:
                    # f16 products (2x; |z*c| <= ~450, no overflow on raw M),
                    # fp32 sums (the cancellation-sensitive part)
                    mul(w[:, 0:3, :], z[:, 0:3, :], c[:, 0:3, :])
                    add(dd[ch], w[:, 0, :], w[:, 1, :])
                    add(dd[ch], dd[ch], w[:, 2, :])
                    act(sg[ch], dd[ch], "Sign")
                    act(dd[ch], dd[ch], "Square")
                else:
                    nc.vector.tensor_scalar(out=w[:, 0:3, :], in0=z[:, 0:3, :],
                                            scalar1=2.0**-8, scalar2=None,
                                            op0=Alu.mult)
                    mul(w[:, 3:6, :], w[:, 0:3, :], c[:, 0:3, :])
                    add(w[:, 6, :], w[:, 3, :], w[:, 4, :])
                    add(w[:, 7, :], w[:, 6, :], w[:, 5, :])
                    act(sg[ch], w[:, 7, :], "Sign")
                    act(dd[ch], w[:, 7, :], "Square", scale=256.0)
                act(dd[ch], dd[ch], "Ln", bias=_c(nc, EPS_D))
                if last:
                    act(bb[ch], dd[ch], "Exp", scale=-1.0/3.0, bias=ofs)
                act(aa[ch], dd[ch], "Exp", scale=-1.0/6.0, bias=ofs)

            def update(ch, it):
                last = it == K_ITERS - 1
                z, c = Z[ch], C[ch]
                mul(sg[ch], sg[ch], aa[ch])         # sg <- a16 (signed)
                zf = z.rearrange("p c n -> p (c n)")
                cf = c.rearrange("p c n -> p (c n)")
                if not last:
                    # drift update: Z <- Z + a*C (scale absorbed by next det)
                    mul(c[:, :, :], c[:, :, :], _bc(sg[ch], 9))
                    add(zf, zf, cf)
                else:
                    mul(W[:, :, :], z[:, :, :], _bc(sg[ch], 9))
                    mul(c[:, :, :], c[:, :, :], _bc(bb[ch], 9))
                    add(zf, W.rearrange("p c n -> p (c n)"), cf)

            def rot(ch, buf, col, halves=1):
                # clamp junk rows, diff pred vs target halves, ACT sq-accum
                z = Z[ch]
                zf = z.rearrange("p c n -> p (c n)")
                nc.vector.tensor_scalar(out=zf, in0=zf, scalar1=8.0,
                                        scalar2=-8.0, op0=Alu.min, op1=Alu.max)
                for h in range(halves):
                    c0, c1 = h * 9 // halves, (h + 1) * 9 // halves
                    sub(buf[:, c0:c1, 0:T], z[:, c0:c1, 0:T],
                        z[:, c0:c1, T:L])
                    act(buf[:, c0:c1, 0:T], buf[:, c0:c1, 0:T], "Square",
                        accum_out=acc[:, col+h:col+h+1])

            # ---- software-pipelined schedule over the two chunks ----
            load(0)
            load(1)
            deint(0, dve_half=True)
            mse(0)
            deint(1)
            cof_det_act(0, 0)
            mse(1)
            cof_det_act(1, 0)
            update(0, 0)
            update(1, 0)
            for it in range(1, K_ITERS):
                cof_det_act(0, it)
                cof_det_act(1, it)
                update(0, it)
                if it == K_ITERS - 1:
                    rot(0, D, 2)        # chunk-0 rot overlaps chunk-1 update
                update(1, it)
            rot(1, W, 3, halves=2)      # split so ACT accum overlaps the sub
            nc.sync.dma_start(out=out[:, :], in_=acc)
    return nc


def _elide_implied_waits(nc):
    """Drop semaphore waits already implied by program order or transitively
    by earlier waits (vector-clock propagation).  Tile's per-instruction wait
    emission is not transitively minimal, and walrus can encode only one sync
    wait on Activation/DMA instructions (and ~4 on control instructions), so
    the redundant waits both break codegen and waste sequencer time.

    Model: each semaphore s carries a snapshot VC at every increment value;
    an engine's observed VC advances via its own instruction stream and via
    the snapshots of the waits it executes.  A wait (s >= v) is dropped iff
    the engine's observed VC already dominates it.  Unknown update modes
    disable elision for that semaphore (conservative).
    """
    join = lambda a, b: {k: max(a.get(k, 0), b.get(k, 0)) for k in set(a) | set(b)}
    sem_val = {}        # sem name -> current value
    sem_snap = {}       # sem name -> list of (value, VC) snapshots
    eng_vc = {}         # engine name -> observed VC
    unsafe = set()      # sems with non-increment updates
    n_drop = 0
    for f in nc.m.functions:
        for bb in f.blocks:
            for ins in bb.instructions:
                eng = str(ins.engine)
                vc = dict(eng_vc.get(eng, {}))
                si = ins.sync_info
                waits = list(si.on_wait) if si is not None and si.on_wait else []
                kept = []
                for w in waits:
                    s, v = w.ant_name, w.wait_value
                    if w.wait_mode != "sem-ge-imm" or s in unsafe:
                        kept.append(w)
                        continue
                    if vc.get(s, 0) >= v:
                        n_drop += 1
                        continue
                    if sem_val.get(s, 0) < v:
                        kept.append(w)
                        continue
                    kept.append(w)
                    snap = {}
                    for sv, svc in sem_snap.get(s, ()):
                        if sv <= v:
                            snap = svc
                        else:
                            break
                    vc = join(vc, snap)
                    vc[s] = max(vc.get(s, 0), v)
                if si is not None and len(kept) != len(waits):
                    si.on_wait = kept
                ups = si.on_update if si is not None and si.on_update else []
                for u in ups:
                    s = u.ant_name
                    if u.update_mode not in ("sem-inc", "sem-add-imm"):
                        unsafe.add(s)
                        continue
                    nv = sem_val.get(s, 0) + (u.update_value or 1)
                    sem_val[s] = nv
                    lst = sem_snap.setdefault(s, [])
                    prev = lst[-1][1] if lst else {}
                    lst.append((nv, join(prev, vc)))
                    if "DMA" not in s:
                        vc[s] = max(vc.get(s, 0), nv)
                eng_vc[eng] = vc
    return n_drop


_NC_CACHE = None


def kernel(pred: np.ndarray, target: np.ndarray) -> np.ndarray:
    global _NC_CACHE
    from concourse.bass_utils import run_bass_kernel_spmd

    pred = np.ascontiguousarray(np.asarray(pred, dtype=np.float32))
    target = np.ascontiguousarray(np.asarray(target, dtype=np.float32))
    assert pred.shape == (B, 9) and target.shape == (B, 9)

    if _NC_CACHE is None:
        _NC_CACHE = _build_nc()
        _elide_implied_waits(_NC_CACHE)
    nc = _NC_CACHE

    ps = pred.reshape(N_CORES, ROWS_PER_CORE, 9)
    ts = target.reshape(N_CORES, ROWS_PER_CORE, 9)
    in_maps = [{"pred": ps[i], "target": ts[i]} for i in range(N_CORES)]
    res = run_bass_kernel_spmd(nc, in_maps, core_ids=list(range(N_CORES)))
    globals()["_LAST_RESULT"] = res

    mse_sum = 0.0
    rot_sum = 0.0
    for r in res.results:
        part = np.asarray(r["partials"], dtype=np.float64)
        mse_sum += part[:, :2].sum()
        rot_sum += part[:, 2:].sum()
    n = float(B * 9)
    return np.asarray(np.float32(mse_sum / n + 0.5 * (rot_sum / n)))
